# revision 40
# baseline (speedup 1.0000x reference)
# Multi-head attention (B=2, S=2048, D=1024, H=16, head_dim=64) with bool mask,
# sharded across 8 TRN2 NeuronCores: core c -> batch c//4, heads 4*(c%4)..4*(c%4)+3.
#
# Per-core device kernel:
#   scoresT = K @ Q^T                 (PE bf16, [128 k, 1024 q] units)
#   eviction of each psS unit to bf16 attn, split 4 ways to balance engines:
#     'A': ACT exp(scale=1/8) -> DVE mask multiply
#     'P': ACT exp(scale=1/8) -> Pool (gpsimd) mask multiply
#     'Z': one fused DVE scalar_tensor_tensor: i16 <- (psS + B') * m'[k,q],
#          bit-reinterpreted as bf16 == Schraudolph exp(s/8) with the mask
#          folded in. The mask tile holds {A'=23.125, 0}; on the A/P paths the
#          same tile is a plain multiplicative mask whose uniform A' factor
#          cancels in the softmax normalization. B' is tuned so the Z path's
#          mean scale matches the A/P paths' A'*exp(s/8) exactly.
#     'I': additive fp8e5 mask folded into PSUM on the otherwise-idle PE (a
#          DoubleRow identity matmul adds -1280 to masked entries), then a
#          mask-free ACT exp with bias ln(A').
#   AV in direct layout: out[q,d] = attnT^T @ [V|1] per 128-q chunk (PE bf16,
#   full 128 output partitions; column 64 is the softmax denominator Z).
#   normalize: DVE reciprocal + broadcast multiply, assembled in SBUF, DMA out.
#
# Host side (inside kernel()): slice per-core shards, pre-transpose Q/K per head
# ([64, S] head-dim-major, bf16), pre-bake the inverted mask transposed as
# {A', 0} bf16, reassemble the 8 per-core bf16 outputs into [B, S, D] f32.

import sys

import numpy as np

for _p in ("/opt/trn_rl_repo",):
    if _p not in sys.path:
        sys.path.insert(0, _p)

import ml_dtypes

import concourse.bass as bass  # noqa: F401  (engine types reachable via nc)
import concourse.tile as tile
from concourse import bacc, mybir
from concourse.masks import make_identity

F32 = mybir.dt.float32
BF16 = mybir.dt.bfloat16
I16 = mybir.dt.int16
FP8E5 = mybir.dt.float8e5

S = 2048          # sequence length
HD = 64           # head dim
HPC = 4           # heads per core
NCORES = 8
B = 2
H = 16
D = H * HD

# Schraudolph constants for the Z path. A' is the exact bf16 rounding of
# 128/(8*ln2); B' is tuned (float32, truncating i16 cast) so that
# E[bitcast_bf16(i16((s+B')*A'))] == A' * exp(s/8) over the score distribution.
A_PRIME = 23.125
B_PRIME = 727.746979

# Optional debug map: instruction name -> semantic label (filled when
# DEBUG_LABELS is a dict; costs nothing when None).
DEBUG_LABELS = None


def _dbg(ins, label):
    if DEBUG_LABELS is not None and ins is not None:
        try:
            DEBUG_LABELS[ins.ins.name] = label
        except AttributeError:
            pass

# Per-phase eviction path patterns (16 k-strip units per phase), alternating.
# Single-engine paths: Z = fused DVE bit-trick, W = same on Pool (gpsimd
# STT), A = ACT exp + DVE mask, P = ACT exp + Pool mask (legacy).
# Split paths (two engines, by column halves - frees the psS PSUM slot in
# ~1.0us, under the 3-slot recycle slack of ~1.28us):
#   S = DVE STT on cols [0:H) + Pool STT on [H:QG)
#   T = ACT exp + DVE mask on [0:H) + Pool STT on [H:QG)
# Per-phase cadence is ~10.3us (QK+AV on PE). 6A+4S+6T:
# ACT 9.4us, DVE 8.1us, Pool 9.3us (incl normalize on Pool).
# One pattern per phase (8 phases at s=2048).
# Phase 0 is mask-DMA-gated: I units take a 1-byte fp8 additive mask (half
# the early DMA bytes) and burn idle PE/ACT time instead.
# Phases 1-6: the baseline alternating A/Z/P mix.
# Phase 7 feeds the tail: S units (split DVE+Pool fused STT) evict fast so
# the final AV/finalize chain drains early.
import os

if os.environ.get("K_MID", "base") == "T":
    # P retired: A7/Z4/T3/W2 per phase. ACT ~8.7, DVE ~9.7, Pool ~7.0
    # (norm on Pool) vs cadence 10.3.
    _B0 = "AWZATAZTAWZATZAA"
    _B1 = "ATZAWAZTATZAWZAA"
    _PH7D = "ASZATAZTASZATZAA"
else:
    _B0 = "AZAPAZPAZPAZPZAZ"
    _B1 = "PAZPAZPAZPAZPAZA"
    _PH7D = _B1
_PH0 = os.environ.get("K_PH0", _B0)
_PH7 = os.environ.get("K_PH7", _PH7D)
PATTERNS = [_PH0, _B0, _B1, _B0, _B1, _B0, _B1, _PH7]
PATTERNS_SMALL = ["WAZP", "SITZ"]
TAIL_FE = int(os.environ.get("K_TAIL_FE", "4"))
AV_START = int(os.environ.get("K_AV_START", "4"))
NORM_POOL = os.environ.get("K_NORM", "dve") == "pool"
# The fused psO-divide finalize races with psO slot reuse under the tile
# scheduler (CoreSim NaN-poisons it); keep the two-op recip+mul finalize.
USE_DIV = os.environ.get("K_DIV", "0") == "1"
N_WARM = int(os.environ.get("K_WARM", "24"))
MS_GPSIMD = os.environ.get("K_MS", "dve") == "gp"
MASK_BIAS = -1280.0  # e5m2-exact; exp((s-1280)/8) == 0 for masked entries


def build_program(s=S, reps=1, patterns=PATTERNS):
    """Build the single-core SPMD program. Returns the compiled Bacc object.

    reps>1 emits the whole body that many times in one NEFF - used to measure
    device time by wall-clock differencing."""
    nc = bacc.Bacc()

    if s < 2048 and patterns is PATTERNS:
        patterns = PATTERNS_SMALL
    KS = s // 128            # number of k strips
    QG = min(1024, s)        # q width of one eviction unit
    NQG = s // QG            # q groups ("halves" at s=2048)
    NCH = QG // 128          # AV q-chunks per group
    CPG = min(4, NCH)        # chunks per psO group

    qkT_d = nc.declare_dram_parameter("qkT", [2, HPC * HD, s], BF16, isOutput=False)
    v_d = nc.declare_dram_parameter(
        "v", [s, HPC * (HD + 1)], BF16, isOutput=False
    )
    nmT_d = nc.declare_dram_parameter("nmT", [s, s], BF16, isOutput=False)
    nm8_d = nc.declare_dram_parameter("nm8", [s, s], FP8E5, isOutput=False)
    out_d = nc.declare_dram_parameter("out", [s, HPC * HD], BF16, isOutput=True)

    # Which mask formats each (g, ks) slot needs, from the per-phase paths:
    # A/Z use the bf16 multiplicative mask, P the int16 AND-mask, I the fp8
    # additive mask. Only the needed pieces are DMA'd / kept resident.
    def slot_paths(g, ks):
        return {
            patterns[(g * HPC + h) % len(patterns)][
                ks % len(patterns[(g * HPC + h) % len(patterns)])]
            for h in range(HPC)
        }

    gks = [(g, ks) for g in range(NQG) for ks in range(KS)]
    i_pieces = sorted(t for t in gks if "I" in slot_paths(*t))
    az_pieces = {t for t in gks if slot_paths(*t) & {"A", "Z", "W", "P", "S", "T"}}

    nm_view = nmT_d[:].rearrange("(ks p) q -> p ks q", p=128)
    nm8_view = nm8_d[:].rearrange("(ks p) q -> p ks q", p=128)
    v_view = v_d[:].rearrange("(ks p) c -> p ks c", p=128)
    out_view = out_d[:].rearrange("(sq p) c -> p sq c", p=128)

    with tile.TileContext(nc) as tc:
        with (
            tc.tile_pool(name="const", bufs=1) as const,
            tc.tile_pool(name="wq", bufs=1) as wq,
            tc.tile_pool(name="attn", bufs=min(2 * KS + 4, 36)) as apool,
            tc.tile_pool(name="stat", bufs=4) as spool,
            tc.tile_pool(name="oasm", bufs=1) as opool,
            tc.tile_pool(name="psS", bufs=3, space="PSUM") as psS_pool,
            tc.tile_pool(name="psO", bufs=2, space="PSUM") as psO_pool,
        ):
            aux = nc.gpsimd if MS_GPSIMD else nc.vector

            # Preload the exp table (emitted before any real exp; runs while
            # the first DMAs stream).
            warm = const.tile([128, 1], F32)
            aux.memset(warm, 0.0)
            nc.scalar.activation(warm, warm, mybir.ActivationFunctionType.Exp)

            # fp8e5 identity for the I-path mask-add matmul (tile 1 unused;
            # the [128, 2, 128] shape + memset keep the original const-setup
            # op stream, whose scheduling the rest of the kernel is tuned to).
            identf = const.tile([128, 128], F32)
            make_identity(nc, identf)
            ident8 = const.tile([128, 2, 128], FP8E5)
            aux.memset(ident8, 0.0)
            aux.tensor_copy(out=ident8[:, 0, :], in_=identf)
            # Per-partition bias ln(A') for the I path's exp.
            lnap = const.tile([128, 1], F32)
            aux.memset(lnap, float(np.log(A_PRIME)))

            # Warm the PE clock (cost model p-state ramp) while input DMAs
            # stream: ~3us of dummy matmuls.
            zb = const.tile([128, 128], BF16)
            nc.vector.memset(zb, 0.0)
            for _ in range(N_WARM):
                wmm = psS_pool.tile([128, QG], F32, tag="psS")
                nc.tensor.matmul(
                    wmm[:, :128], lhsT=zb[0:64, :], rhs=zb[0:64, :],
                    start=True, stop=True,
                )

            def qk_src(pair):
                return qkT_d[:, 128 * pair:128 * pair + 128, :].rearrange(
                    "t p s -> p t s"
                )

            def emit_body():
                # Q^T / K^T head pairs: [128, 2, s] (head 2p on partitions
                # 0-63, head 2p+1 on 64-127; dim1: 0=Q^T, 1=K^T).
                qks = []
                for pair in range(HPC // 2):
                    qk = wq.tile([128, 2, s], BF16, tag=f"qkT{pair}")
                    qks.append(qk)
                v_sb = wq.tile([128, KS, HPC * (HD + 1)], BF16, tag="vsb")
                nm_sb = wq.tile([128, KS, s], BF16, tag="nm")
                KH = KS // 2
                # All input DMAs ride the SP HWDGE queue (SP has no compute,
                # so ring-full stalls never block a compute sequencer; gpsimd
                # dma_start is SWDGE and would burn Pool engine time). Pieces
                # are ordered by first use; phases run q-group-major, so mask
                # q-group 1 is not needed until ~halfway through the kernel.
                nm8p = {}
                for (g, ks) in i_pieces:
                    t = wq.tile([128, QG], FP8E5, tag=f"nm8_{g}_{ks}",
                                name=f"nm8_{g}_{ks}")
                    nm8p[(g, ks)] = t

                def nm_piece(ks, g):
                    if (g, ks) in az_pieces:
                        nc.sync.dma_start(
                            out=nm_sb[:, ks, g * QG:(g + 1) * QG],
                            in_=nm_view[:, ks, g * QG:(g + 1) * QG],
                        )

                def nm8_piece(ks, g):
                    nc.sync.dma_start(
                        out=nm8p[(g, ks)],
                        in_=nm8_view[:, ks, g * QG:(g + 1) * QG],
                    )

                # First Q/K pair split by head (partition halves) so head 0's
                # slices land in ~a quarter of the full-pair DMA time.
                nc.scalar.dma_start(
                    out=qks[0][0:HD, 0, :], in_=qk_src(0)[0:HD, 0, :]
                )
                nc.sync.dma_start(
                    out=qks[0][0:HD, 1, :], in_=qk_src(0)[0:HD, 1, :]
                )
                nc.scalar.dma_start(
                    out=qks[0][HD:, 0, :], in_=qk_src(0)[HD:, 0, :]
                )
                nc.sync.dma_start(
                    out=qks[0][HD:, 1, :], in_=qk_src(0)[HD:, 1, :]
                )
                # Phase-0 mask pieces in consumption order: I strips need only
                # the 1-byte fp8 piece now (bf16 copy deferred until after v).
                ph0 = patterns[0]
                deferred = []
                for ks in range(KS):
                    if ph0[ks % len(ph0)] == "I":
                        if (0, ks) in i_pieces:
                            nm8_piece(ks, 0)
                        if (0, ks) in az_pieces:
                            deferred.append(("nm", ks))
                    else:
                        if (0, ks) in az_pieces:
                            nm_piece(ks, 0)
                        if (0, ks) in i_pieces:
                            deferred.append(("nm8", ks))
                nc.sync.dma_start(out=v_sb[:, :KH], in_=v_view[:, :KH])
                nc.sync.dma_start(out=v_sb[:, KH:], in_=v_view[:, KH:])
                for kind, ks in deferred:
                    (nm_piece if kind == "nm" else nm8_piece)(ks, 0)
                for pair in range(1, HPC // 2):
                    nc.sync.dma_start(out=qks[pair], in_=qk_src(pair))
                for g in range(1, NQG):
                    for ks in range(KS):
                        if (g, ks) in i_pieces:
                            nm8_piece(ks, g)
                        if (g, ks) in az_pieces:
                            nm_piece(ks, g)


                out_asm = opool.tile([128, KS, HPC * HD], BF16)

                # q-group-major phase order: the first HPC phases only touch
                # mask q-group 0, giving the mask DMA stream headroom.
                phases = [(h, g) for g in range(NQG) for h in range(HPC)]

                def emit_av_chunk(ph, c, av_state):
                    """AV matmuls for q-chunk c of phase ph, plus group
                    finalize (reciprocal + normalize) every CPG chunks.

                    Strips are read in eviction-completion order (Z first,
                    then A, then P): the last strips read are the ones whose
                    masks lag past the phase boundary, so the PE never waits
                    on a straggling Pool/DVE mask with work still in hand."""
                    h, g = ph
                    ats = av_state["ats"]
                    order = av_state["order"]
                    if c % CPG == 0:
                        av_state["psO"] = psO_pool.tile(
                            [128, CPG, 128], F32, tag="psO", name="psO"
                        )
                    psO = av_state["psO"]
                    for i, ks in enumerate(order):
                        _dbg(nc.tensor.matmul(
                            psO[:, c % CPG, 0:HD + 1],
                            lhsT=ats[ks][:, c * 128:(c + 1) * 128],
                            rhs=v_sb[:, ks, h * (HD + 1):(h + 1) * (HD + 1)],
                            start=(i == 0),
                            stop=(i == KS - 1),
                        ), f"AV h{h}g{g} c{c} ks{ks}")
                    fe = av_state.get("fin_every", CPG)
                    if c % fe == fe - 1:
                        fin = (h, g, c - (fe - 1), fe, psO)
                        if c == NCH - 1 and av_state.get("defer_last"):
                            av_state["deferred"] = fin
                        else:
                            emit_finalize(fin, av_state.get("norm_engine"))

                def emit_finalize(fin, norm_engine=None):
                    h, g, c0, fe, psO = fin
                    qc0 = g * NCH + c0
                    eng = norm_engine if norm_engine is not None else (
                        nc.gpsimd if NORM_POOL else nc.vector)
                    if USE_DIV:
                        # Single fused normalize: out = psO[:, :, 0:HD] / Z
                        # (column HD broadcast) - no separate reciprocal.
                        _dbg(eng.tensor_tensor(
                            out_asm[:, qc0:qc0 + fe, h * HD:(h + 1) * HD],
                            psO[:, c0 % CPG:c0 % CPG + fe, 0:HD],
                            psO[:, c0 % CPG:c0 % CPG + fe,
                                HD:HD + 1].to_broadcast([128, fe, HD]),
                            mybir.AluOpType.divide,
                        ), f"norm h{h}g{g} c{c0}")
                        return
                    rec = spool.tile([128, fe], F32, tag="rec", name="rec")
                    _dbg(nc.vector.reciprocal(
                        rec, psO[:, c0 % CPG:c0 % CPG + fe, HD]),
                         f"recip h{h}g{g} c{c0}")
                    _dbg(eng.tensor_mul(
                        out_asm[:, qc0:qc0 + fe, h * HD:(h + 1) * HD],
                        psO[:, c0 % CPG:c0 % CPG + fe, 0:HD],
                        rec.to_broadcast([128, fe, HD]),
                    ), f"norm h{h}g{g} c{c0}")
                    if h == HPC - 1:
                        nc.sync.dma_start(
                            out=out_view[:, qc0:qc0 + fe, :],
                            in_=out_asm[:, qc0:qc0 + fe, :],
                        )

                prev = None  # (phase, {"ats": [...]}) awaiting AV
                pending_fin = []
                unit = 0  # global eviction-unit counter (for path pattern)
                for ph in phases:
                    h, g = ph
                    base = HD * (h % 2)
                    pair = h // 2
                    q0 = g * QG
                    ats = []
                    paths = []
                    for ks in range(KS):
                        # QK for this unit
                        pat = patterns[(unit // KS) % len(patterns)]
                        path = pat[ks % len(pat)]
                        psS = psS_pool.tile([128, QG], F32, tag="psS")
                        for qc in range(QG // 512):
                            _dbg(nc.tensor.matmul(
                                psS[:, qc * 512:(qc + 1) * 512],
                                lhsT=qks[pair][base:base + HD, 1,
                                               ks * 128:(ks + 1) * 128],
                                rhs=qks[pair][base:base + HD, 0,
                                              q0 + qc * 512:q0 + (qc + 1) * 512],
                                start=True,
                                stop=(path != "I"),
                            ), f"QK h{h}g{g} ks{ks}")
                        if path == "I":
                            # Mask-add on the PE: psS += I^T @ nm8 via a plain
                            # fp8 matmul (~213ns per 512 columns).
                            for qc in range(QG // 512):
                                _dbg(nc.tensor.matmul(
                                    psS[:, qc * 512:(qc + 1) * 512],
                                    lhsT=ident8[:, 0, :],
                                    rhs=nm8p[(g, ks)][:,
                                                      qc * 512:(qc + 1) * 512],
                                    start=False,
                                    stop=True,
                                ), f"maskI h{h}g{g} ks{ks}")
                        # Chunks ride units 4..~12: late enough that the
                        # previous phase's last evictions have drained, early
                        # enough that attn slots recycle before phase p+2.
                        if ks == 1 and pending_fin:
                            emit_finalize(pending_fin.pop())
                        if prev is not None:
                            start = AV_START if KS > 8 else 1
                            den = max(KS - start - 2, 1)
                            for c in range(NCH):
                                if min(start + c * den // NCH, KS - 1) == ks:
                                    emit_av_chunk(prev[0], c, prev[1])
                        # Eviction: psS -> masked bf16 attn tile
                        at = apool.tile([128, QG], BF16, tag="at")
                        nm_slice = nm_sb[:, ks, q0:q0 + QG]
                        unit += 1
                        if path == "I":
                            # Mask already added in PSUM; exp with bias ln(A')
                            # scales the weights to match the other paths.
                            _dbg(nc.scalar.activation(
                                at, psS, mybir.ActivationFunctionType.Exp,
                                scale=0.125, bias=lnap[:],
                            ), f"expI h{h}g{g} ks{ks}")
                        elif path == "Z":
                            _dbg(nc.vector.scalar_tensor_tensor(
                                at[:].bitcast(I16),
                                psS[:],
                                B_PRIME,
                                nm_slice,
                                mybir.AluOpType.add,
                                mybir.AluOpType.mult,
                            ), f"STT h{h}g{g} ks{ks}")
                        elif path == "W":
                            _dbg(nc.gpsimd.scalar_tensor_tensor(
                                at[:].bitcast(I16),
                                psS[:],
                                B_PRIME,
                                nm_slice,
                                mybir.AluOpType.add,
                                mybir.AluOpType.mult,
                            ), f"STTW h{h}g{g} ks{ks}")
                        elif path in ("S", "T"):
                            # Split eviction: fast engines take cols [0:HF)
                            # (read by AV chunks 0..NCH/2-1, emitted first);
                            # Pool STT takes [HF:QG).
                            HF = QG // 2
                            if path == "S":
                                _dbg(nc.vector.scalar_tensor_tensor(
                                    at[:, 0:HF].bitcast(I16),
                                    psS[:, 0:HF],
                                    B_PRIME,
                                    nm_slice[:, 0:HF],
                                    mybir.AluOpType.add,
                                    mybir.AluOpType.mult,
                                ), f"STTh h{h}g{g} ks{ks}")
                            else:
                                _dbg(nc.scalar.activation(
                                    at[:, 0:HF], psS[:, 0:HF],
                                    mybir.ActivationFunctionType.Exp,
                                    scale=0.125,
                                ), f"expTh h{h}g{g} ks{ks}")
                                _dbg(nc.vector.tensor_mul(
                                    at[:, 0:HF], at[:, 0:HF],
                                    nm_slice[:, 0:HF]),
                                    f"maskTh h{h}g{g} ks{ks}")
                            _dbg(nc.gpsimd.scalar_tensor_tensor(
                                at[:, HF:QG].bitcast(I16),
                                psS[:, HF:QG],
                                B_PRIME,
                                nm_slice[:, HF:QG],
                                mybir.AluOpType.add,
                                mybir.AluOpType.mult,
                            ), f"STTWh h{h}g{g} ks{ks}")
                        elif path == "A":
                            _dbg(nc.scalar.activation(
                                at, psS, mybir.ActivationFunctionType.Exp,
                                scale=0.125,
                            ), f"expA h{h}g{g} ks{ks}")
                            _dbg(nc.vector.tensor_mul(at, at, nm_slice),
                                 f"maskA h{h}g{g} ks{ks}")
                        else:
                            _dbg(nc.scalar.activation(
                                at, psS, mybir.ActivationFunctionType.Exp,
                                scale=0.125,
                            ), f"expP h{h}g{g} ks{ks}")
                            _dbg(nc.gpsimd.tensor_mul(at, at, nm_slice),
                                 f"maskP h{h}g{g} ks{ks}")
                        ats.append(at)
                        paths.append(path)
                    rank = {"I": 0, "Z": 1, "S": 2, "T": 3, "A": 4, "W": 5,
                            "P": 6}
                    order = sorted(range(KS), key=lambda k: (rank[paths[k]], k))
                    if prev is not None and prev[1].get("deferred"):
                        pending_fin.append(prev[1]["deferred"])
                    prev = (ph, {"ats": ats, "order": order,
                                 "defer_last": KS > 8})
                # Tail: AV of the final phase. Finalize every 2 chunks (and
                # DMA out per finalize) so the drain chain after the last AV
                # matmul is short.
                prev[1]["defer_last"] = False
                prev[1]["fin_every"] = TAIL_FE
                for fin in pending_fin:
                    emit_finalize(fin)
                for c in range(NCH):
                    emit_av_chunk(prev[0], c, prev[1])

            for _ in range(reps):
                emit_body()
    nc.compile()
    return nc


_CACHE = {}


def _get_nc():
    if "nc" not in _CACHE:
        _CACHE["nc"] = build_program()
    return _CACHE["nc"]


def make_in_maps(q, k, v, mask, s=S):
    """Shard full inputs into 8 per-core input maps (host-side layout prep)."""
    q = np.asarray(q, dtype=np.float32)
    k = np.asarray(k, dtype=np.float32)
    v = np.asarray(v, dtype=np.float32)
    mask = np.asarray(mask)
    nh = q.shape[-1] // HD
    in_maps = []
    for c in range(NCORES):
        b, g = divmod(c, NCORES // B)
        h0 = HPC * g
        qs = q[b].reshape(s, nh, HD)[:, h0:h0 + HPC, :]      # [s, HPC, 64]
        ks_ = k[b].reshape(s, nh, HD)[:, h0:h0 + HPC, :]
        qkT = np.empty((2, HPC * HD, s), ml_dtypes.bfloat16)
        qkT[0] = qs.transpose(1, 2, 0).reshape(HPC * HD, s)
        qkT[1] = ks_.transpose(1, 2, 0).reshape(HPC * HD, s)
        vh = v[b, :, h0 * HD:(h0 + HPC) * HD].reshape(s, HPC, HD)
        vc = np.concatenate(
            [vh, np.ones((s, HPC, 1), np.float32)], axis=2
        ).reshape(s, HPC * (HD + 1)).astype(ml_dtypes.bfloat16)
        mT = mask[b].T
        nmT = (np.float32(A_PRIME) * (~mT).astype(np.float32)).astype(
            ml_dtypes.bfloat16
        )
        nm8 = (np.float32(MASK_BIAS) * mT.astype(np.float32)).astype(
            ml_dtypes.float8_e5m2
        )
        in_maps.append({"qkT": qkT, "v": vc, "nmT": nmT, "nm8": nm8})
    return in_maps


def assemble_out(results, s=S, d=D):
    out = np.empty((B, s, d), np.float32)
    for c in range(NCORES):
        b, g = divmod(c, NCORES // B)
        out[b, :, g * HPC * HD:(g + 1) * HPC * HD] = results[c]["out"]
    return out


def kernel(q, k, v, mask):
    from concourse.bass_utils import run_bass_kernel_spmd

    nc = _get_nc()
    in_maps = make_in_maps(q, k, v, mask)
    res = run_bass_kernel_spmd(nc, in_maps, list(range(NCORES))).results
    return assemble_out(res)



# revision 41
# speedup vs baseline: 1.0025x; 1.0025x over previous
# Multi-head attention (B=2, S=2048, D=1024, H=16, head_dim=64) with bool mask,
# sharded across 8 TRN2 NeuronCores: core c -> batch c//4, heads 4*(c%4)..4*(c%4)+3.
#
# Per-core device kernel:
#   scoresT = K @ Q^T                 (PE bf16, [128 k, 1024 q] units)
#   eviction of each psS unit to bf16 attn, split 4 ways to balance engines:
#     'A': ACT exp(scale=1/8) -> DVE mask multiply
#     'P': ACT exp(scale=1/8) -> Pool (gpsimd) mask multiply
#     'Z': one fused DVE scalar_tensor_tensor: i16 <- (psS + B') * m'[k,q],
#          bit-reinterpreted as bf16 == Schraudolph exp(s/8) with the mask
#          folded in. The mask tile holds {A'=23.125, 0}; on the A/P paths the
#          same tile is a plain multiplicative mask whose uniform A' factor
#          cancels in the softmax normalization. B' is tuned so the Z path's
#          mean scale matches the A/P paths' A'*exp(s/8) exactly.
#     'I': additive fp8e5 mask folded into PSUM on the otherwise-idle PE (a
#          DoubleRow identity matmul adds -1280 to masked entries), then a
#          mask-free ACT exp with bias ln(A').
#   AV in direct layout: out[q,d] = attnT^T @ [V|1] per 128-q chunk (PE bf16,
#   full 128 output partitions; column 64 is the softmax denominator Z).
#   normalize: DVE reciprocal + broadcast multiply, assembled in SBUF, DMA out.
#
# Host side (inside kernel()): slice per-core shards, pre-transpose Q/K per head
# ([64, S] head-dim-major, bf16), pre-bake the inverted mask transposed as
# {A', 0} bf16, reassemble the 8 per-core bf16 outputs into [B, S, D] f32.

import sys

import numpy as np

for _p in ("/opt/trn_rl_repo",):
    if _p not in sys.path:
        sys.path.insert(0, _p)

import ml_dtypes

import concourse.bass as bass  # noqa: F401  (engine types reachable via nc)
import concourse.tile as tile
from concourse import bacc, mybir
from concourse.masks import make_identity

F32 = mybir.dt.float32
BF16 = mybir.dt.bfloat16
I16 = mybir.dt.int16
FP8E5 = mybir.dt.float8e5

S = 2048          # sequence length
HD = 64           # head dim
HPC = 4           # heads per core
NCORES = 8
B = 2
H = 16
D = H * HD

# Schraudolph constants for the Z path. A' is the exact bf16 rounding of
# 128/(8*ln2); B' is tuned (float32, truncating i16 cast) so that
# E[bitcast_bf16(i16((s+B')*A'))] == A' * exp(s/8) over the score distribution.
A_PRIME = 23.125
B_PRIME = 727.746979

# Optional debug map: instruction name -> semantic label (filled when
# DEBUG_LABELS is a dict; costs nothing when None).
DEBUG_LABELS = None


def _dbg(ins, label):
    if DEBUG_LABELS is not None and ins is not None:
        try:
            DEBUG_LABELS[ins.ins.name] = label
        except AttributeError:
            pass

# Per-phase eviction path patterns (16 k-strip units per phase), alternating.
# Single-engine paths: Z = fused DVE bit-trick, W = same on Pool (gpsimd
# STT), A = ACT exp + DVE mask, P = ACT exp + Pool mask (legacy).
# Split paths (two engines, by column halves - frees the psS PSUM slot in
# ~1.0us, under the 3-slot recycle slack of ~1.28us):
#   S = DVE STT on cols [0:H) + Pool STT on [H:QG)
#   T = ACT exp + DVE mask on [0:H) + Pool STT on [H:QG)
# Per-phase cadence is ~10.3us (QK+AV on PE). 6A+4S+6T:
# ACT 9.4us, DVE 8.1us, Pool 9.3us (incl normalize on Pool).
# One pattern per phase (8 phases at s=2048).
# Phase 0 is mask-DMA-gated: I units take a 1-byte fp8 additive mask (half
# the early DMA bytes) and burn idle PE/ACT time instead.
# Phases 1-6: the baseline alternating A/Z/P mix.
# Phase 7 feeds the tail: S units (split DVE+Pool fused STT) evict fast so
# the final AV/finalize chain drains early.
import os

if os.environ.get("K_MID", "base") == "T":
    # P retired: A7/Z4/T3/W2 per phase. ACT ~8.7, DVE ~9.7, Pool ~7.0
    # (norm on Pool) vs cadence 10.3.
    _B0 = "AWZATAZTAWZATZAA"
    _B1 = "ATZAWAZTATZAWZAA"
    _PH7D = "ASZATAZTASZATZAA"
else:
    _B0 = "AZAPAZPAZPAZPZAZ"
    _B1 = "PAZPAZPAZPAZPAZA"
    _PH7D = _B1
_PH0 = os.environ.get("K_PH0", _B0)
_PH7 = os.environ.get("K_PH7", _PH7D)
PATTERNS = [_PH0, _B1, _B0, _B1, _B0, _B1, _B0, _PH7]
PATTERNS_SMALL = ["WAZP", "SITZ"]
TAIL_FE = int(os.environ.get("K_TAIL_FE", "4"))
AV_START = int(os.environ.get("K_AV_START", "4"))
NORM_POOL = os.environ.get("K_NORM", "dve") == "pool"
# The fused psO-divide finalize races with psO slot reuse under the tile
# scheduler (CoreSim NaN-poisons it); keep the two-op recip+mul finalize.
USE_DIV = os.environ.get("K_DIV", "0") == "1"
N_WARM = int(os.environ.get("K_WARM", "24"))
MS_GPSIMD = os.environ.get("K_MS", "dve") == "gp"
MASK_BIAS = -1280.0  # e5m2-exact; exp((s-1280)/8) == 0 for masked entries


def build_program(s=S, reps=1, patterns=PATTERNS):
    """Build the single-core SPMD program. Returns the compiled Bacc object.

    reps>1 emits the whole body that many times in one NEFF - used to measure
    device time by wall-clock differencing."""
    nc = bacc.Bacc()

    if s < 2048 and patterns is PATTERNS:
        patterns = PATTERNS_SMALL
    KS = s // 128            # number of k strips
    QG = min(1024, s)        # q width of one eviction unit
    NQG = s // QG            # q groups ("halves" at s=2048)
    NCH = QG // 128          # AV q-chunks per group
    CPG = min(4, NCH)        # chunks per psO group

    qkT_d = nc.declare_dram_parameter("qkT", [2, HPC * HD, s], BF16, isOutput=False)
    v_d = nc.declare_dram_parameter(
        "v", [s, HPC * (HD + 1)], BF16, isOutput=False
    )
    nmT_d = nc.declare_dram_parameter("nmT", [s, s], BF16, isOutput=False)
    nm8_d = nc.declare_dram_parameter("nm8", [s, s], FP8E5, isOutput=False)
    out_d = nc.declare_dram_parameter("out", [s, HPC * HD], BF16, isOutput=True)

    # Which mask formats each (g, ks) slot needs, from the per-phase paths:
    # A/Z use the bf16 multiplicative mask, P the int16 AND-mask, I the fp8
    # additive mask. Only the needed pieces are DMA'd / kept resident.
    def slot_paths(g, ks):
        return {
            patterns[(g * HPC + h) % len(patterns)][
                ks % len(patterns[(g * HPC + h) % len(patterns)])]
            for h in range(HPC)
        }

    gks = [(g, ks) for g in range(NQG) for ks in range(KS)]
    i_pieces = sorted(t for t in gks if "I" in slot_paths(*t))
    az_pieces = {t for t in gks if slot_paths(*t) & {"A", "Z", "W", "P", "S", "T"}}

    nm_view = nmT_d[:].rearrange("(ks p) q -> p ks q", p=128)
    nm8_view = nm8_d[:].rearrange("(ks p) q -> p ks q", p=128)
    v_view = v_d[:].rearrange("(ks p) c -> p ks c", p=128)
    out_view = out_d[:].rearrange("(sq p) c -> p sq c", p=128)

    with tile.TileContext(nc) as tc:
        with (
            tc.tile_pool(name="const", bufs=1) as const,
            tc.tile_pool(name="wq", bufs=1) as wq,
            tc.tile_pool(name="attn", bufs=min(2 * KS + 4, 36)) as apool,
            tc.tile_pool(name="stat", bufs=4) as spool,
            tc.tile_pool(name="oasm", bufs=1) as opool,
            tc.tile_pool(name="psS", bufs=3, space="PSUM") as psS_pool,
            tc.tile_pool(name="psO", bufs=2, space="PSUM") as psO_pool,
        ):
            aux = nc.gpsimd if MS_GPSIMD else nc.vector

            # Preload the exp table (emitted before any real exp; runs while
            # the first DMAs stream).
            warm = const.tile([128, 1], F32)
            aux.memset(warm, 0.0)
            nc.scalar.activation(warm, warm, mybir.ActivationFunctionType.Exp)

            # fp8e5 identity for the I-path mask-add matmul (tile 1 unused;
            # the [128, 2, 128] shape + memset keep the original const-setup
            # op stream, whose scheduling the rest of the kernel is tuned to).
            identf = const.tile([128, 128], F32)
            make_identity(nc, identf)
            ident8 = const.tile([128, 2, 128], FP8E5)
            aux.memset(ident8, 0.0)
            aux.tensor_copy(out=ident8[:, 0, :], in_=identf)
            # Per-partition bias ln(A') for the I path's exp.
            lnap = const.tile([128, 1], F32)
            aux.memset(lnap, float(np.log(A_PRIME)))

            # Warm the PE clock (cost model p-state ramp) while input DMAs
            # stream: ~3us of dummy matmuls.
            zb = const.tile([128, 128], BF16)
            nc.vector.memset(zb, 0.0)
            for _ in range(N_WARM):
                wmm = psS_pool.tile([128, QG], F32, tag="psS")
                nc.tensor.matmul(
                    wmm[:, :128], lhsT=zb[0:64, :], rhs=zb[0:64, :],
                    start=True, stop=True,
                )

            def qk_src(pair):
                return qkT_d[:, 128 * pair:128 * pair + 128, :].rearrange(
                    "t p s -> p t s"
                )

            def emit_body():
                # Q^T / K^T head pairs: [128, 2, s] (head 2p on partitions
                # 0-63, head 2p+1 on 64-127; dim1: 0=Q^T, 1=K^T).
                qks = []
                for pair in range(HPC // 2):
                    qk = wq.tile([128, 2, s], BF16, tag=f"qkT{pair}")
                    qks.append(qk)
                v_sb = wq.tile([128, KS, HPC * (HD + 1)], BF16, tag="vsb")
                nm_sb = wq.tile([128, KS, s], BF16, tag="nm")
                KH = KS // 2
                # All input DMAs ride the SP HWDGE queue (SP has no compute,
                # so ring-full stalls never block a compute sequencer; gpsimd
                # dma_start is SWDGE and would burn Pool engine time). Pieces
                # are ordered by first use; phases run q-group-major, so mask
                # q-group 1 is not needed until ~halfway through the kernel.
                nm8p = {}
                for (g, ks) in i_pieces:
                    t = wq.tile([128, QG], FP8E5, tag=f"nm8_{g}_{ks}",
                                name=f"nm8_{g}_{ks}")
                    nm8p[(g, ks)] = t

                def nm_piece(ks, g):
                    if (g, ks) in az_pieces:
                        nc.sync.dma_start(
                            out=nm_sb[:, ks, g * QG:(g + 1) * QG],
                            in_=nm_view[:, ks, g * QG:(g + 1) * QG],
                        )

                def nm8_piece(ks, g):
                    nc.sync.dma_start(
                        out=nm8p[(g, ks)],
                        in_=nm8_view[:, ks, g * QG:(g + 1) * QG],
                    )

                # First Q/K pair split by head (partition halves) so head 0's
                # slices land in ~a quarter of the full-pair DMA time.
                nc.scalar.dma_start(
                    out=qks[0][0:HD, 0, :], in_=qk_src(0)[0:HD, 0, :]
                )
                nc.sync.dma_start(
                    out=qks[0][0:HD, 1, :], in_=qk_src(0)[0:HD, 1, :]
                )
                nc.scalar.dma_start(
                    out=qks[0][HD:, 0, :], in_=qk_src(0)[HD:, 0, :]
                )
                nc.sync.dma_start(
                    out=qks[0][HD:, 1, :], in_=qk_src(0)[HD:, 1, :]
                )
                # Phase-0 mask pieces in consumption order: I strips need only
                # the 1-byte fp8 piece now (bf16 copy deferred until after v).
                ph0 = patterns[0]
                deferred = []
                for ks in range(KS):
                    if ph0[ks % len(ph0)] == "I":
                        if (0, ks) in i_pieces:
                            nm8_piece(ks, 0)
                        if (0, ks) in az_pieces:
                            deferred.append(("nm", ks))
                    else:
                        if (0, ks) in az_pieces:
                            nm_piece(ks, 0)
                        if (0, ks) in i_pieces:
                            deferred.append(("nm8", ks))
                nc.sync.dma_start(out=v_sb[:, :KH], in_=v_view[:, :KH])
                nc.sync.dma_start(out=v_sb[:, KH:], in_=v_view[:, KH:])
                for kind, ks in deferred:
                    (nm_piece if kind == "nm" else nm8_piece)(ks, 0)
                for pair in range(1, HPC // 2):
                    nc.sync.dma_start(out=qks[pair], in_=qk_src(pair))
                for g in range(1, NQG):
                    for ks in range(KS):
                        if (g, ks) in i_pieces:
                            nm8_piece(ks, g)
                        if (g, ks) in az_pieces:
                            nm_piece(ks, g)


                out_asm = opool.tile([128, KS, HPC * HD], BF16)

                # q-group-major phase order: the first HPC phases only touch
                # mask q-group 0, giving the mask DMA stream headroom.
                phases = [(h, g) for g in range(NQG) for h in range(HPC)]

                def emit_av_chunk(ph, c, av_state):
                    """AV matmuls for q-chunk c of phase ph, plus group
                    finalize (reciprocal + normalize) every CPG chunks.

                    Strips are read in eviction-completion order (Z first,
                    then A, then P): the last strips read are the ones whose
                    masks lag past the phase boundary, so the PE never waits
                    on a straggling Pool/DVE mask with work still in hand."""
                    h, g = ph
                    ats = av_state["ats"]
                    order = av_state["order"]
                    if c % CPG == 0:
                        av_state["psO"] = psO_pool.tile(
                            [128, CPG, 128], F32, tag="psO", name="psO"
                        )
                    psO = av_state["psO"]
                    for i, ks in enumerate(order):
                        _dbg(nc.tensor.matmul(
                            psO[:, c % CPG, 0:HD + 1],
                            lhsT=ats[ks][:, c * 128:(c + 1) * 128],
                            rhs=v_sb[:, ks, h * (HD + 1):(h + 1) * (HD + 1)],
                            start=(i == 0),
                            stop=(i == KS - 1),
                        ), f"AV h{h}g{g} c{c} ks{ks}")
                    fe = av_state.get("fin_every", CPG)
                    if c % fe == fe - 1:
                        fin = (h, g, c - (fe - 1), fe, psO)
                        if c == NCH - 1 and av_state.get("defer_last"):
                            av_state["deferred"] = fin
                        else:
                            emit_finalize(fin, av_state.get("norm_engine"))

                def emit_finalize(fin, norm_engine=None):
                    h, g, c0, fe, psO = fin
                    qc0 = g * NCH + c0
                    eng = norm_engine if norm_engine is not None else (
                        nc.gpsimd if NORM_POOL else nc.vector)
                    if USE_DIV:
                        # Single fused normalize: out = psO[:, :, 0:HD] / Z
                        # (column HD broadcast) - no separate reciprocal.
                        _dbg(eng.tensor_tensor(
                            out_asm[:, qc0:qc0 + fe, h * HD:(h + 1) * HD],
                            psO[:, c0 % CPG:c0 % CPG + fe, 0:HD],
                            psO[:, c0 % CPG:c0 % CPG + fe,
                                HD:HD + 1].to_broadcast([128, fe, HD]),
                            mybir.AluOpType.divide,
                        ), f"norm h{h}g{g} c{c0}")
                        return
                    rec = spool.tile([128, fe], F32, tag="rec", name="rec")
                    _dbg(nc.vector.reciprocal(
                        rec, psO[:, c0 % CPG:c0 % CPG + fe, HD]),
                         f"recip h{h}g{g} c{c0}")
                    _dbg(eng.tensor_mul(
                        out_asm[:, qc0:qc0 + fe, h * HD:(h + 1) * HD],
                        psO[:, c0 % CPG:c0 % CPG + fe, 0:HD],
                        rec.to_broadcast([128, fe, HD]),
                    ), f"norm h{h}g{g} c{c0}")
                    if h == HPC - 1:
                        nc.sync.dma_start(
                            out=out_view[:, qc0:qc0 + fe, :],
                            in_=out_asm[:, qc0:qc0 + fe, :],
                        )

                prev = None  # (phase, {"ats": [...]}) awaiting AV
                pending_fin = []
                unit = 0  # global eviction-unit counter (for path pattern)
                for ph in phases:
                    h, g = ph
                    base = HD * (h % 2)
                    pair = h // 2
                    q0 = g * QG
                    ats = []
                    paths = []
                    for ks in range(KS):
                        # QK for this unit
                        pat = patterns[(unit // KS) % len(patterns)]
                        path = pat[ks % len(pat)]
                        psS = psS_pool.tile([128, QG], F32, tag="psS")
                        for qc in range(QG // 512):
                            _dbg(nc.tensor.matmul(
                                psS[:, qc * 512:(qc + 1) * 512],
                                lhsT=qks[pair][base:base + HD, 1,
                                               ks * 128:(ks + 1) * 128],
                                rhs=qks[pair][base:base + HD, 0,
                                              q0 + qc * 512:q0 + (qc + 1) * 512],
                                start=True,
                                stop=(path != "I"),
                            ), f"QK h{h}g{g} ks{ks}")
                        if path == "I":
                            # Mask-add on the PE: psS += I^T @ nm8 via a plain
                            # fp8 matmul (~213ns per 512 columns).
                            for qc in range(QG // 512):
                                _dbg(nc.tensor.matmul(
                                    psS[:, qc * 512:(qc + 1) * 512],
                                    lhsT=ident8[:, 0, :],
                                    rhs=nm8p[(g, ks)][:,
                                                      qc * 512:(qc + 1) * 512],
                                    start=False,
                                    stop=True,
                                ), f"maskI h{h}g{g} ks{ks}")
                        # Chunks ride units 4..~12: late enough that the
                        # previous phase's last evictions have drained, early
                        # enough that attn slots recycle before phase p+2.
                        if ks == 1 and pending_fin:
                            emit_finalize(pending_fin.pop())
                        if prev is not None:
                            start = AV_START if KS > 8 else 1
                            den = max(KS - start - 2, 1)
                            for c in range(NCH):
                                if min(start + c * den // NCH, KS - 1) == ks:
                                    emit_av_chunk(prev[0], c, prev[1])
                        # Eviction: psS -> masked bf16 attn tile
                        at = apool.tile([128, QG], BF16, tag="at")
                        nm_slice = nm_sb[:, ks, q0:q0 + QG]
                        unit += 1
                        if path == "I":
                            # Mask already added in PSUM; exp with bias ln(A')
                            # scales the weights to match the other paths.
                            _dbg(nc.scalar.activation(
                                at, psS, mybir.ActivationFunctionType.Exp,
                                scale=0.125, bias=lnap[:],
                            ), f"expI h{h}g{g} ks{ks}")
                        elif path == "Z":
                            _dbg(nc.vector.scalar_tensor_tensor(
                                at[:].bitcast(I16),
                                psS[:],
                                B_PRIME,
                                nm_slice,
                                mybir.AluOpType.add,
                                mybir.AluOpType.mult,
                            ), f"STT h{h}g{g} ks{ks}")
                        elif path == "W":
                            _dbg(nc.gpsimd.scalar_tensor_tensor(
                                at[:].bitcast(I16),
                                psS[:],
                                B_PRIME,
                                nm_slice,
                                mybir.AluOpType.add,
                                mybir.AluOpType.mult,
                            ), f"STTW h{h}g{g} ks{ks}")
                        elif path in ("S", "T"):
                            # Split eviction: fast engines take cols [0:HF)
                            # (read by AV chunks 0..NCH/2-1, emitted first);
                            # Pool STT takes [HF:QG).
                            HF = QG // 2
                            if path == "S":
                                _dbg(nc.vector.scalar_tensor_tensor(
                                    at[:, 0:HF].bitcast(I16),
                                    psS[:, 0:HF],
                                    B_PRIME,
                                    nm_slice[:, 0:HF],
                                    mybir.AluOpType.add,
                                    mybir.AluOpType.mult,
                                ), f"STTh h{h}g{g} ks{ks}")
                            else:
                                _dbg(nc.scalar.activation(
                                    at[:, 0:HF], psS[:, 0:HF],
                                    mybir.ActivationFunctionType.Exp,
                                    scale=0.125,
                                ), f"expTh h{h}g{g} ks{ks}")
                                _dbg(nc.vector.tensor_mul(
                                    at[:, 0:HF], at[:, 0:HF],
                                    nm_slice[:, 0:HF]),
                                    f"maskTh h{h}g{g} ks{ks}")
                            _dbg(nc.gpsimd.scalar_tensor_tensor(
                                at[:, HF:QG].bitcast(I16),
                                psS[:, HF:QG],
                                B_PRIME,
                                nm_slice[:, HF:QG],
                                mybir.AluOpType.add,
                                mybir.AluOpType.mult,
                            ), f"STTWh h{h}g{g} ks{ks}")
                        elif path == "A":
                            _dbg(nc.scalar.activation(
                                at, psS, mybir.ActivationFunctionType.Exp,
                                scale=0.125,
                            ), f"expA h{h}g{g} ks{ks}")
                            _dbg(nc.vector.tensor_mul(at, at, nm_slice),
                                 f"maskA h{h}g{g} ks{ks}")
                        else:
                            _dbg(nc.scalar.activation(
                                at, psS, mybir.ActivationFunctionType.Exp,
                                scale=0.125,
                            ), f"expP h{h}g{g} ks{ks}")
                            _dbg(nc.gpsimd.tensor_mul(at, at, nm_slice),
                                 f"maskP h{h}g{g} ks{ks}")
                        ats.append(at)
                        paths.append(path)
                    rank = {"I": 0, "Z": 1, "S": 2, "T": 3, "A": 4, "W": 5,
                            "P": 6}
                    order = sorted(range(KS), key=lambda k: (rank[paths[k]], k))
                    if prev is not None and prev[1].get("deferred"):
                        pending_fin.append(prev[1]["deferred"])
                    prev = (ph, {"ats": ats, "order": order,
                                 "defer_last": KS > 8})
                # Tail: AV of the final phase. Finalize every 2 chunks (and
                # DMA out per finalize) so the drain chain after the last AV
                # matmul is short.
                prev[1]["defer_last"] = False
                prev[1]["fin_every"] = TAIL_FE
                for fin in pending_fin:
                    emit_finalize(fin)
                for c in range(NCH):
                    emit_av_chunk(prev[0], c, prev[1])

            for _ in range(reps):
                emit_body()
    nc.compile()
    return nc


_CACHE = {}


def _get_nc():
    if "nc" not in _CACHE:
        _CACHE["nc"] = build_program()
    return _CACHE["nc"]


def make_in_maps(q, k, v, mask, s=S):
    """Shard full inputs into 8 per-core input maps (host-side layout prep)."""
    q = np.asarray(q, dtype=np.float32)
    k = np.asarray(k, dtype=np.float32)
    v = np.asarray(v, dtype=np.float32)
    mask = np.asarray(mask)
    nh = q.shape[-1] // HD
    in_maps = []
    for c in range(NCORES):
        b, g = divmod(c, NCORES // B)
        h0 = HPC * g
        qs = q[b].reshape(s, nh, HD)[:, h0:h0 + HPC, :]      # [s, HPC, 64]
        ks_ = k[b].reshape(s, nh, HD)[:, h0:h0 + HPC, :]
        qkT = np.empty((2, HPC * HD, s), ml_dtypes.bfloat16)
        qkT[0] = qs.transpose(1, 2, 0).reshape(HPC * HD, s)
        qkT[1] = ks_.transpose(1, 2, 0).reshape(HPC * HD, s)
        vh = v[b, :, h0 * HD:(h0 + HPC) * HD].reshape(s, HPC, HD)
        vc = np.concatenate(
            [vh, np.ones((s, HPC, 1), np.float32)], axis=2
        ).reshape(s, HPC * (HD + 1)).astype(ml_dtypes.bfloat16)
        mT = mask[b].T
        nmT = (np.float32(A_PRIME) * (~mT).astype(np.float32)).astype(
            ml_dtypes.bfloat16
        )
        nm8 = (np.float32(MASK_BIAS) * mT.astype(np.float32)).astype(
            ml_dtypes.float8_e5m2
        )
        in_maps.append({"qkT": qkT, "v": vc, "nmT": nmT, "nm8": nm8})
    return in_maps


def assemble_out(results, s=S, d=D):
    out = np.empty((B, s, d), np.float32)
    for c in range(NCORES):
        b, g = divmod(c, NCORES // B)
        out[b, :, g * HPC * HD:(g + 1) * HPC * HD] = results[c]["out"]
    return out


def kernel(q, k, v, mask):
    from concourse.bass_utils import run_bass_kernel_spmd

    nc = _get_nc()
    in_maps = make_in_maps(q, k, v, mask)
    res = run_bass_kernel_spmd(nc, in_maps, list(range(NCORES))).results
    return assemble_out(res)



# revision 55
# speedup vs baseline: 1.0102x; 1.0076x over previous
# Multi-head attention (B=2, S=2048, D=1024, H=16, head_dim=64) with bool mask,
# sharded across 8 TRN2 NeuronCores: core c -> batch c//4, heads 4*(c%4)..4*(c%4)+3.
#
# Per-core device kernel:
#   scoresT = K @ Q^T                 (PE bf16, [128 k, 1024 q] units)
#   eviction of each psS unit to bf16 attn, split 4 ways to balance engines:
#     'A': ACT exp(scale=1/8) -> DVE mask multiply
#     'P': ACT exp(scale=1/8) -> Pool (gpsimd) mask multiply
#     'Z': one fused DVE scalar_tensor_tensor: i16 <- (psS + B') * m'[k,q],
#          bit-reinterpreted as bf16 == Schraudolph exp(s/8) with the mask
#          folded in. The mask tile holds {A'=23.125, 0}; on the A/P paths the
#          same tile is a plain multiplicative mask whose uniform A' factor
#          cancels in the softmax normalization. B' is tuned so the Z path's
#          mean scale matches the A/P paths' A'*exp(s/8) exactly.
#     'I': additive fp8e5 mask folded into PSUM on the otherwise-idle PE (a
#          DoubleRow identity matmul adds -1280 to masked entries), then a
#          mask-free ACT exp with bias ln(A').
#   AV in direct layout: out[q,d] = attnT^T @ [V|1] per 128-q chunk (PE bf16,
#   full 128 output partitions; column 64 is the softmax denominator Z).
#   normalize: DVE reciprocal + broadcast multiply, assembled in SBUF, DMA out.
#
# Host side (inside kernel()): slice per-core shards, pre-transpose Q/K per head
# ([64, S] head-dim-major, bf16), pre-bake the inverted mask transposed as
# {A', 0} bf16, reassemble the 8 per-core bf16 outputs into [B, S, D] f32.

import sys

import numpy as np

for _p in ("/opt/trn_rl_repo",):
    if _p not in sys.path:
        sys.path.insert(0, _p)

import ml_dtypes

import concourse.bass as bass  # noqa: F401  (engine types reachable via nc)
import concourse.tile as tile
from concourse import bacc, mybir
from concourse.masks import make_identity

F32 = mybir.dt.float32
BF16 = mybir.dt.bfloat16
I16 = mybir.dt.int16
FP8E5 = mybir.dt.float8e5
FP8E4 = mybir.dt.float8e4

S = 2048          # sequence length
HD = 64           # head dim
HPC = 4           # heads per core
NCORES = 8
B = 2
H = 16
D = H * HD

# Schraudolph constants for the Z path. A' is the exact bf16 rounding of
# 128/(8*ln2); B' is tuned (float32, truncating i16 cast) so that
# E[bitcast_bf16(i16((s+B')*A'))] == A' * exp(s/8) over the score distribution.
A_PRIME = 23.125
B_PRIME = 727.746979

# Optional debug map: instruction name -> semantic label (filled when
# DEBUG_LABELS is a dict; costs nothing when None).
DEBUG_LABELS = None


def _dbg(ins, label):
    if DEBUG_LABELS is not None and ins is not None:
        try:
            DEBUG_LABELS[ins.ins.name] = label
        except AttributeError:
            pass

# Per-phase eviction path patterns (16 k-strip units per phase), alternating.
# Single-engine paths: Z = fused DVE bit-trick, W = same on Pool (gpsimd
# STT), A = ACT exp + DVE mask, P = ACT exp + Pool mask (legacy).
# Split paths (two engines, by column halves - frees the psS PSUM slot in
# ~1.0us, under the 3-slot recycle slack of ~1.28us):
#   S = DVE STT on cols [0:H) + Pool STT on [H:QG)
#   T = ACT exp + DVE mask on [0:H) + Pool STT on [H:QG)
# Per-phase cadence is ~10.3us (QK+AV on PE). 6A+4S+6T:
# ACT 9.4us, DVE 8.1us, Pool 9.3us (incl normalize on Pool).
# One pattern per phase (8 phases at s=2048).
# Phase 0 is mask-DMA-gated: I units take a 1-byte fp8 additive mask (half
# the early DMA bytes) and burn idle PE/ACT time instead.
# Phases 1-6: the baseline alternating A/Z/P mix.
# Phase 7 feeds the tail: S units (split DVE+Pool fused STT) evict fast so
# the final AV/finalize chain drains early.
import os

if os.environ.get("K_MID", "base") == "T":
    # P retired: A7/Z4/T3/W2 per phase. ACT ~8.7, DVE ~9.7, Pool ~7.0
    # (norm on Pool) vs cadence 10.3.
    _B0 = "AWZATAZTAWZATZAA"
    _B1 = "ATZAWAZTATZAWZAA"
    _PH7D = "ASZATAZTASZATZAA"
else:
    _B0 = "AZAPAZPAZPAZPZAZ"
    _B1 = "PAZPAZPAZPAZPAZA"
    _PH7D = _B1
_B0 = os.environ.get("K_B0", _B0)
_B1 = os.environ.get("K_B1", _B1)
_PH0 = os.environ.get("K_PH0", _B0)
_PH7 = os.environ.get("K_PH7", os.environ.get("K_B1", _PH7D))
PATTERNS = [_PH0, _B1, _B0, _B1, _B0, _B1, _B0, _PH7]
PATTERNS_SMALL = ["WAZP", "TIZP"]  # ks3 is pure-P -> covers the fp8 P mask
TAIL_FE = int(os.environ.get("K_TAIL_FE", "4"))
AV_START = int(os.environ.get("K_AV_START", "4"))
NORM_POOL = os.environ.get("K_NORM", "dve") == "pool"
# The fused psO-divide finalize races with psO slot reuse under the tile
# scheduler (CoreSim NaN-poisons it); keep the two-op recip+mul finalize.
USE_DIV = os.environ.get("K_DIV", "0") == "1"
N_WARM = int(os.environ.get("K_WARM", "24"))
MS_GPSIMD = os.environ.get("K_MS", "dve") == "gp"
MASK_BIAS = -1280.0  # e5m2-exact; exp((s-1280)/8) == 0 for masked entries


def build_program(s=S, reps=1, patterns=PATTERNS):
    """Build the single-core SPMD program. Returns the compiled Bacc object.

    reps>1 emits the whole body that many times in one NEFF - used to measure
    device time by wall-clock differencing."""
    nc = bacc.Bacc()

    if s < 2048 and patterns is PATTERNS:
        patterns = PATTERNS_SMALL
    KS = s // 128            # number of k strips
    QG = min(1024, s)        # q width of one eviction unit
    NQG = s // QG            # q groups ("halves" at s=2048)
    NCH = QG // 128          # AV q-chunks per group
    CPG = min(4, NCH)        # chunks per psO group

    qkT_d = nc.declare_dram_parameter("qkT", [2, HPC * HD, s], BF16, isOutput=False)
    v_d = nc.declare_dram_parameter(
        "v", [s, HPC * (HD + 1)], BF16, isOutput=False
    )
    nmT_d = nc.declare_dram_parameter("nmT", [s, s], BF16, isOutput=False)
    nm8_d = nc.declare_dram_parameter("nm8", [s, s], FP8E5, isOutput=False)
    nm8m_d = nc.declare_dram_parameter("nm8m", [s, s], FP8E4, isOutput=False)
    out_d = nc.declare_dram_parameter("out", [s, HPC * HD], BF16, isOutput=True)

    # Which mask formats each (g, ks) slot needs, from the per-phase paths:
    # A/Z use the bf16 multiplicative mask, P the int16 AND-mask, I the fp8
    # additive mask. Only the needed pieces are DMA'd / kept resident.
    def slot_paths(g, ks):
        return {
            patterns[(g * HPC + h) % len(patterns)][
                ks % len(patterns[(g * HPC + h) % len(patterns)])]
            for h in range(HPC)
        }

    gks = [(g, ks) for g in range(NQG) for ks in range(KS)]
    i_pieces = sorted(t for t in gks if "I" in slot_paths(*t))
    # Strips that are P-path in EVERY phase can take a 1-byte fp8 {1,0}
    # multiplicative mask: the x A' scale folds into the ACT exp bias
    # (ln A'), and Pool's tensor_mul has no dtype-dependent cost. Same
    # instructions, same emission order - only the DMA stream shrinks.
    fp8m_pieces = {t for t in gks if slot_paths(*t) == {"P"}}
    az_pieces = {t for t in gks
                 if (slot_paths(*t) & {"A", "Z", "W", "P", "S", "T"})
                 and t not in fp8m_pieces}

    nm_view = nmT_d[:].rearrange("(ks p) q -> p ks q", p=128)
    nm8_view = nm8_d[:].rearrange("(ks p) q -> p ks q", p=128)
    nm8m_view = nm8m_d[:].rearrange("(ks p) q -> p ks q", p=128)
    v_view = v_d[:].rearrange("(ks p) c -> p ks c", p=128)
    out_view = out_d[:].rearrange("(sq p) c -> p sq c", p=128)

    with tile.TileContext(nc) as tc:
        with (
            tc.tile_pool(name="const", bufs=1) as const,
            tc.tile_pool(name="wq", bufs=1) as wq,
            tc.tile_pool(name="attn", bufs=min(2 * KS + 4, 36)) as apool,
            tc.tile_pool(name="stat", bufs=4) as spool,
            tc.tile_pool(name="oasm", bufs=1) as opool,
            tc.tile_pool(name="psS", bufs=3, space="PSUM") as psS_pool,
            tc.tile_pool(name="psO", bufs=2, space="PSUM") as psO_pool,
        ):
            aux = nc.gpsimd if MS_GPSIMD else nc.vector

            if os.environ.get("K_ZB", "late") == "first":
                zb = const.tile([128, 128], BF16)
                nc.vector.memset(zb, 0.0)

            # Preload the exp table (emitted before any real exp; runs while
            # the first DMAs stream).
            warm = const.tile([128, 1], F32)
            aux.memset(warm, 0.0)
            nc.scalar.activation(warm, warm, mybir.ActivationFunctionType.Exp)

            # fp8e5 identity for the I-path mask-add matmul (tile 1 unused;
            # the [128, 2, 128] shape + memset keep the original const-setup
            # op stream, whose scheduling the rest of the kernel is tuned to).
            identf = const.tile([128, 128], F32)
            make_identity(nc, identf)
            ident8 = const.tile([128, 2, 128], FP8E5)
            aux.memset(ident8, 0.0)
            aux.tensor_copy(out=ident8[:, 0, :], in_=identf)
            # Per-partition bias ln(A') for the I path's exp.
            lnap = const.tile([128, 1], F32)
            aux.memset(lnap, float(np.log(A_PRIME)))

            # Warm the PE clock (cost model p-state ramp) while input DMAs
            # stream: ~3us of dummy matmuls.
            if os.environ.get("K_ZB", "late") == "first":
                pass
            else:
                zb = const.tile([128, 128], BF16)
                nc.vector.memset(zb, 0.0)
            for _ in range(N_WARM):
                wmm = psS_pool.tile([128, QG], F32, tag="psS")
                nc.tensor.matmul(
                    wmm[:, :128], lhsT=zb[0:64, :], rhs=zb[0:64, :],
                    start=True, stop=True,
                )

            def qk_src(pair):
                return qkT_d[:, 128 * pair:128 * pair + 128, :].rearrange(
                    "t p s -> p t s"
                )

            def emit_body():
                # Q^T / K^T head pairs: [128, 2, s] (head 2p on partitions
                # 0-63, head 2p+1 on 64-127; dim1: 0=Q^T, 1=K^T).
                qks = []
                for pair in range(HPC // 2):
                    qk = wq.tile([128, 2, s], BF16, tag=f"qkT{pair}")
                    qks.append(qk)
                v_sb = wq.tile([128, KS, HPC * (HD + 1)], BF16, tag="vsb")
                nm_sb = wq.tile([128, KS, s], BF16, tag="nm")
                KH = KS // 2
                # All input DMAs ride the SP HWDGE queue (SP has no compute,
                # so ring-full stalls never block a compute sequencer; gpsimd
                # dma_start is SWDGE and would burn Pool engine time). Pieces
                # are ordered by first use; phases run q-group-major, so mask
                # q-group 1 is not needed until ~halfway through the kernel.
                nm8p = {}
                for (g, ks) in i_pieces:
                    t = wq.tile([128, QG], FP8E5, tag=f"nm8_{g}_{ks}",
                                name=f"nm8_{g}_{ks}")
                    nm8p[(g, ks)] = t
                nm8mp = {}
                for (g, ks) in sorted(fp8m_pieces):
                    t = wq.tile([128, QG], FP8E4, tag=f"nm8m_{g}_{ks}",
                                name=f"nm8m_{g}_{ks}")
                    nm8mp[(g, ks)] = t

                def nm_piece(ks, g):
                    if (g, ks) in fp8m_pieces:
                        nc.sync.dma_start(
                            out=nm8mp[(g, ks)],
                            in_=nm8m_view[:, ks, g * QG:(g + 1) * QG],
                        )
                    elif (g, ks) in az_pieces:
                        nc.sync.dma_start(
                            out=nm_sb[:, ks, g * QG:(g + 1) * QG],
                            in_=nm_view[:, ks, g * QG:(g + 1) * QG],
                        )

                def nm8_piece(ks, g):
                    nc.sync.dma_start(
                        out=nm8p[(g, ks)],
                        in_=nm8_view[:, ks, g * QG:(g + 1) * QG],
                    )

                # First Q/K pair split by head (partition halves) so head 0's
                # slices land in ~a quarter of the full-pair DMA time.
                nc.scalar.dma_start(
                    out=qks[0][0:HD, 0, :], in_=qk_src(0)[0:HD, 0, :]
                )
                nc.sync.dma_start(
                    out=qks[0][0:HD, 1, :], in_=qk_src(0)[0:HD, 1, :]
                )
                nc.scalar.dma_start(
                    out=qks[0][HD:, 0, :], in_=qk_src(0)[HD:, 0, :]
                )
                nc.sync.dma_start(
                    out=qks[0][HD:, 1, :], in_=qk_src(0)[HD:, 1, :]
                )
                # Phase-0 mask pieces in consumption order: I strips need only
                # the 1-byte fp8 piece now (bf16 copy deferred until after v).
                ph0 = patterns[0]
                deferred = []
                for ks in range(KS):
                    if ph0[ks % len(ph0)] == "I":
                        if (0, ks) in i_pieces:
                            nm8_piece(ks, 0)
                        if (0, ks) in az_pieces or (0, ks) in fp8m_pieces:
                            deferred.append(("nm", ks))
                    else:
                        if (0, ks) in az_pieces or (0, ks) in fp8m_pieces:
                            nm_piece(ks, 0)
                        if (0, ks) in i_pieces:
                            deferred.append(("nm8", ks))
                nc.sync.dma_start(out=v_sb[:, :KH], in_=v_view[:, :KH])
                nc.sync.dma_start(out=v_sb[:, KH:], in_=v_view[:, KH:])
                for kind, ks in deferred:
                    (nm_piece if kind == "nm" else nm8_piece)(ks, 0)
                for pair in range(1, HPC // 2):
                    nc.sync.dma_start(out=qks[pair], in_=qk_src(pair))
                for g in range(1, NQG):
                    for ks in range(KS):
                        if (g, ks) in i_pieces:
                            nm8_piece(ks, g)
                        if (g, ks) in az_pieces or (g, ks) in fp8m_pieces:
                            nm_piece(ks, g)


                out_asm = opool.tile([128, KS, HPC * HD], BF16)

                # q-group-major phase order: the first HPC phases only touch
                # mask q-group 0, giving the mask DMA stream headroom.
                phases = [(h, g) for g in range(NQG) for h in range(HPC)]

                def emit_av_chunk(ph, c, av_state):
                    """AV matmuls for q-chunk c of phase ph, plus group
                    finalize (reciprocal + normalize) every CPG chunks.

                    Strips are read in eviction-completion order (Z first,
                    then A, then P): the last strips read are the ones whose
                    masks lag past the phase boundary, so the PE never waits
                    on a straggling Pool/DVE mask with work still in hand."""
                    h, g = ph
                    ats = av_state["ats"]
                    order = av_state["order"]
                    if c % CPG == 0:
                        av_state["psO"] = psO_pool.tile(
                            [128, CPG, 128], F32, tag="psO", name="psO"
                        )
                    psO = av_state["psO"]
                    for i, ks in enumerate(order):
                        _dbg(nc.tensor.matmul(
                            psO[:, c % CPG, 0:HD + 1],
                            lhsT=ats[ks][:, c * 128:(c + 1) * 128],
                            rhs=v_sb[:, ks, h * (HD + 1):(h + 1) * (HD + 1)],
                            start=(i == 0),
                            stop=(i == KS - 1),
                        ), f"AV h{h}g{g} c{c} ks{ks}")
                    fe = av_state.get("fin_every", CPG)
                    if c % fe == fe - 1:
                        fin = (h, g, c - (fe - 1), fe, psO)
                        if c == NCH - 1 and av_state.get("defer_last"):
                            av_state["deferred"] = fin
                        else:
                            emit_finalize(fin, av_state.get("norm_engine"))

                def emit_finalize(fin, norm_engine=None):
                    h, g, c0, fe, psO = fin
                    qc0 = g * NCH + c0
                    eng = norm_engine if norm_engine is not None else (
                        nc.gpsimd if NORM_POOL else nc.vector)
                    if USE_DIV:
                        # Single fused normalize: out = psO[:, :, 0:HD] / Z
                        # (column HD broadcast) - no separate reciprocal.
                        _dbg(eng.tensor_tensor(
                            out_asm[:, qc0:qc0 + fe, h * HD:(h + 1) * HD],
                            psO[:, c0 % CPG:c0 % CPG + fe, 0:HD],
                            psO[:, c0 % CPG:c0 % CPG + fe,
                                HD:HD + 1].to_broadcast([128, fe, HD]),
                            mybir.AluOpType.divide,
                        ), f"norm h{h}g{g} c{c0}")
                        return
                    rec = spool.tile([128, fe], F32, tag="rec", name="rec")
                    _dbg(nc.vector.reciprocal(
                        rec, psO[:, c0 % CPG:c0 % CPG + fe, HD]),
                         f"recip h{h}g{g} c{c0}")
                    _dbg(eng.tensor_mul(
                        out_asm[:, qc0:qc0 + fe, h * HD:(h + 1) * HD],
                        psO[:, c0 % CPG:c0 % CPG + fe, 0:HD],
                        rec.to_broadcast([128, fe, HD]),
                    ), f"norm h{h}g{g} c{c0}")
                    if h == HPC - 1:
                        nc.sync.dma_start(
                            out=out_view[:, qc0:qc0 + fe, :],
                            in_=out_asm[:, qc0:qc0 + fe, :],
                        )

                prev = None  # (phase, {"ats": [...]}) awaiting AV
                pending_fin = []
                unit = 0  # global eviction-unit counter (for path pattern)
                for ph in phases:
                    h, g = ph
                    base = HD * (h % 2)
                    pair = h // 2
                    q0 = g * QG
                    ats = []
                    paths = []
                    for ks in range(KS):
                        # QK for this unit
                        pat = patterns[(unit // KS) % len(patterns)]
                        path = pat[ks % len(pat)]
                        psS = psS_pool.tile([128, QG], F32, tag="psS")
                        for qc in range(QG // 512):
                            _dbg(nc.tensor.matmul(
                                psS[:, qc * 512:(qc + 1) * 512],
                                lhsT=qks[pair][base:base + HD, 1,
                                               ks * 128:(ks + 1) * 128],
                                rhs=qks[pair][base:base + HD, 0,
                                              q0 + qc * 512:q0 + (qc + 1) * 512],
                                start=True,
                                stop=(path != "I"),
                            ), f"QK h{h}g{g} ks{ks}")
                        if path == "I":
                            # Mask-add on the PE: psS += I^T @ nm8 via a plain
                            # fp8 matmul (~213ns per 512 columns).
                            for qc in range(QG // 512):
                                _dbg(nc.tensor.matmul(
                                    psS[:, qc * 512:(qc + 1) * 512],
                                    lhsT=ident8[:, 0, :],
                                    rhs=nm8p[(g, ks)][:,
                                                      qc * 512:(qc + 1) * 512],
                                    start=False,
                                    stop=True,
                                ), f"maskI h{h}g{g} ks{ks}")
                        # Chunks ride units 4..~12: late enough that the
                        # previous phase's last evictions have drained, early
                        # enough that attn slots recycle before phase p+2.
                        if ks == 1 and pending_fin:
                            emit_finalize(pending_fin.pop())
                        if prev is not None:
                            start = AV_START if KS > 8 else 1
                            den = max(KS - start - 2, 1)
                            for c in range(NCH):
                                if min(start + c * den // NCH, KS - 1) == ks:
                                    emit_av_chunk(prev[0], c, prev[1])
                        # Eviction: psS -> masked bf16 attn tile
                        at = apool.tile([128, QG], BF16, tag="at")
                        nm_slice = nm_sb[:, ks, q0:q0 + QG]
                        unit += 1
                        if path == "I":
                            # Mask already added in PSUM; exp with bias ln(A')
                            # scales the weights to match the other paths.
                            _dbg(nc.scalar.activation(
                                at, psS, mybir.ActivationFunctionType.Exp,
                                scale=0.125, bias=lnap[:],
                            ), f"expI h{h}g{g} ks{ks}")
                        elif path == "Z":
                            _dbg(nc.vector.scalar_tensor_tensor(
                                at[:].bitcast(I16),
                                psS[:],
                                B_PRIME,
                                nm_slice,
                                mybir.AluOpType.add,
                                mybir.AluOpType.mult,
                            ), f"STT h{h}g{g} ks{ks}")
                        elif path == "W":
                            _dbg(nc.gpsimd.scalar_tensor_tensor(
                                at[:].bitcast(I16),
                                psS[:],
                                B_PRIME,
                                nm_slice,
                                mybir.AluOpType.add,
                                mybir.AluOpType.mult,
                            ), f"STTW h{h}g{g} ks{ks}")
                        elif path in ("S", "T"):
                            # Split eviction: fast engines take cols [0:HF)
                            # (read by AV chunks 0..NCH/2-1, emitted first);
                            # Pool STT takes [HF:QG).
                            HF = QG // 2
                            if path == "S":
                                _dbg(nc.vector.scalar_tensor_tensor(
                                    at[:, 0:HF].bitcast(I16),
                                    psS[:, 0:HF],
                                    B_PRIME,
                                    nm_slice[:, 0:HF],
                                    mybir.AluOpType.add,
                                    mybir.AluOpType.mult,
                                ), f"STTh h{h}g{g} ks{ks}")
                            else:
                                _dbg(nc.scalar.activation(
                                    at[:, 0:HF], psS[:, 0:HF],
                                    mybir.ActivationFunctionType.Exp,
                                    scale=0.125,
                                ), f"expTh h{h}g{g} ks{ks}")
                                _dbg(nc.vector.tensor_mul(
                                    at[:, 0:HF], at[:, 0:HF],
                                    nm_slice[:, 0:HF]),
                                    f"maskTh h{h}g{g} ks{ks}")
                            _dbg(nc.gpsimd.scalar_tensor_tensor(
                                at[:, HF:QG].bitcast(I16),
                                psS[:, HF:QG],
                                B_PRIME,
                                nm_slice[:, HF:QG],
                                mybir.AluOpType.add,
                                mybir.AluOpType.mult,
                            ), f"STTWh h{h}g{g} ks{ks}")
                        elif path == "A":
                            _dbg(nc.scalar.activation(
                                at, psS, mybir.ActivationFunctionType.Exp,
                                scale=0.125,
                            ), f"expA h{h}g{g} ks{ks}")
                            _dbg(nc.vector.tensor_mul(at, at, nm_slice),
                                 f"maskA h{h}g{g} ks{ks}")
                        elif (g, ks) in fp8m_pieces:
                            # P-path with 1-byte mask: x A' folded into the
                            # exp bias, Pool multiplies by fp8 {1, 0}.
                            _dbg(nc.scalar.activation(
                                at, psS, mybir.ActivationFunctionType.Exp,
                                scale=0.125, bias=lnap[:],
                            ), f"expP8 h{h}g{g} ks{ks}")
                            _dbg(nc.gpsimd.tensor_mul(
                                at, at, nm8mp[(g, ks)]),
                                 f"maskP8 h{h}g{g} ks{ks}")
                        else:
                            _dbg(nc.scalar.activation(
                                at, psS, mybir.ActivationFunctionType.Exp,
                                scale=0.125,
                            ), f"expP h{h}g{g} ks{ks}")
                            _dbg(nc.gpsimd.tensor_mul(at, at, nm_slice),
                                 f"maskP h{h}g{g} ks{ks}")
                        ats.append(at)
                        paths.append(path)
                    rank = {"I": 0, "Z": 1, "S": 2, "T": 3, "A": 4, "W": 5,
                            "P": 6}
                    order = sorted(range(KS), key=lambda k: (rank[paths[k]], k))
                    if prev is not None and prev[1].get("deferred"):
                        pending_fin.append(prev[1]["deferred"])
                    prev = (ph, {"ats": ats, "order": order,
                                 "defer_last": KS > 8})
                # Tail: AV of the final phase. Finalize every 2 chunks (and
                # DMA out per finalize) so the drain chain after the last AV
                # matmul is short.
                prev[1]["defer_last"] = False
                prev[1]["fin_every"] = TAIL_FE
                for fin in pending_fin:
                    emit_finalize(fin)
                for c in range(NCH):
                    emit_av_chunk(prev[0], c, prev[1])

            for _ in range(reps):
                emit_body()
    nc.compile()
    return nc


_CACHE = {}


def _get_nc():
    if "nc" not in _CACHE:
        _CACHE["nc"] = build_program()
    return _CACHE["nc"]


def make_in_maps(q, k, v, mask, s=S):
    """Shard full inputs into 8 per-core input maps (host-side layout prep)."""
    q = np.asarray(q, dtype=np.float32)
    k = np.asarray(k, dtype=np.float32)
    v = np.asarray(v, dtype=np.float32)
    mask = np.asarray(mask)
    nh = q.shape[-1] // HD
    in_maps = []
    for c in range(NCORES):
        b, g = divmod(c, NCORES // B)
        h0 = HPC * g
        qs = q[b].reshape(s, nh, HD)[:, h0:h0 + HPC, :]      # [s, HPC, 64]
        ks_ = k[b].reshape(s, nh, HD)[:, h0:h0 + HPC, :]
        qkT = np.empty((2, HPC * HD, s), ml_dtypes.bfloat16)
        qkT[0] = qs.transpose(1, 2, 0).reshape(HPC * HD, s)
        qkT[1] = ks_.transpose(1, 2, 0).reshape(HPC * HD, s)
        vh = v[b, :, h0 * HD:(h0 + HPC) * HD].reshape(s, HPC, HD)
        vc = np.concatenate(
            [vh, np.ones((s, HPC, 1), np.float32)], axis=2
        ).reshape(s, HPC * (HD + 1)).astype(ml_dtypes.bfloat16)
        mT = mask[b].T
        nmT = (np.float32(A_PRIME) * (~mT).astype(np.float32)).astype(
            ml_dtypes.bfloat16
        )
        nm8 = (np.float32(MASK_BIAS) * mT.astype(np.float32)).astype(
            ml_dtypes.float8_e5m2
        )
        nm8m = (~mT).astype(np.float32).astype(ml_dtypes.float8_e4m3fn)
        in_maps.append(
            {"qkT": qkT, "v": vc, "nmT": nmT, "nm8": nm8, "nm8m": nm8m}
        )
    return in_maps


def assemble_out(results, s=S, d=D):
    out = np.empty((B, s, d), np.float32)
    for c in range(NCORES):
        b, g = divmod(c, NCORES // B)
        out[b, :, g * HPC * HD:(g + 1) * HPC * HD] = results[c]["out"]
    return out


def kernel(q, k, v, mask):
    from concourse.bass_utils import run_bass_kernel_spmd

    nc = _get_nc()
    in_maps = make_in_maps(q, k, v, mask)
    res = run_bass_kernel_spmd(nc, in_maps, list(range(NCORES))).results
    return assemble_out(res)



# revision 64
# speedup vs baseline: 1.0129x; 1.0027x over previous
# Multi-head attention (B=2, S=2048, D=1024, H=16, head_dim=64) with bool mask,
# sharded across 8 TRN2 NeuronCores: core c -> batch c//4, heads 4*(c%4)..4*(c%4)+3.
#
# Per-core device kernel:
#   scoresT = K @ Q^T                 (PE bf16, [128 k, 1024 q] units)
#   eviction of each psS unit to bf16 attn, split 4 ways to balance engines:
#     'A': ACT exp(scale=1/8) -> DVE mask multiply
#     'P': ACT exp(scale=1/8) -> Pool (gpsimd) mask multiply
#     'Z': one fused DVE scalar_tensor_tensor: i16 <- (psS + B') * m'[k,q],
#          bit-reinterpreted as bf16 == Schraudolph exp(s/8) with the mask
#          folded in. The mask tile holds {A'=23.125, 0}; on the A/P paths the
#          same tile is a plain multiplicative mask whose uniform A' factor
#          cancels in the softmax normalization. B' is tuned so the Z path's
#          mean scale matches the A/P paths' A'*exp(s/8) exactly.
#     'I': additive fp8e5 mask folded into PSUM on the otherwise-idle PE (a
#          DoubleRow identity matmul adds -1280 to masked entries), then a
#          mask-free ACT exp with bias ln(A').
#   AV in direct layout: out[q,d] = attnT^T @ [V|1] per 128-q chunk (PE bf16,
#   full 128 output partitions; column 64 is the softmax denominator Z).
#   normalize: DVE reciprocal + broadcast multiply, assembled in SBUF, DMA out.
#
# Host side (inside kernel()): slice per-core shards, pre-transpose Q/K per head
# ([64, S] head-dim-major, bf16), pre-bake the inverted mask transposed as
# {A', 0} bf16, reassemble the 8 per-core bf16 outputs into [B, S, D] f32.

import sys

import numpy as np

for _p in ("/opt/trn_rl_repo",):
    if _p not in sys.path:
        sys.path.insert(0, _p)

import ml_dtypes

import concourse.bass as bass  # noqa: F401  (engine types reachable via nc)
import concourse.tile as tile
from concourse import bacc, mybir
from concourse.masks import make_identity

F32 = mybir.dt.float32
BF16 = mybir.dt.bfloat16
I16 = mybir.dt.int16
FP8E5 = mybir.dt.float8e5
FP8E4 = mybir.dt.float8e4

S = 2048          # sequence length
HD = 64           # head dim
HPC = 4           # heads per core
NCORES = 8
B = 2
H = 16
D = H * HD

# Schraudolph constants for the Z path. A' is the exact bf16 rounding of
# 128/(8*ln2); B' is tuned (float32, truncating i16 cast) so that
# E[bitcast_bf16(i16((s+B')*A'))] == A' * exp(s/8) over the score distribution.
A_PRIME = 23.125
B_PRIME = 727.746979

# Optional debug map: instruction name -> semantic label (filled when
# DEBUG_LABELS is a dict; costs nothing when None).
DEBUG_LABELS = None


def _dbg(ins, label):
    if DEBUG_LABELS is not None and ins is not None:
        try:
            DEBUG_LABELS[ins.ins.name] = label
        except AttributeError:
            pass

# Per-phase eviction path patterns (16 k-strip units per phase), alternating.
# Single-engine paths: Z = fused DVE bit-trick, W = same on Pool (gpsimd
# STT), A = ACT exp + DVE mask, P = ACT exp + Pool mask (legacy).
# Split paths (two engines, by column halves - frees the psS PSUM slot in
# ~1.0us, under the 3-slot recycle slack of ~1.28us):
#   S = DVE STT on cols [0:H) + Pool STT on [H:QG)
#   T = ACT exp + DVE mask on [0:H) + Pool STT on [H:QG)
# Per-phase cadence is ~10.3us (QK+AV on PE). 6A+4S+6T:
# ACT 9.4us, DVE 8.1us, Pool 9.3us (incl normalize on Pool).
# One pattern per phase (8 phases at s=2048).
# Phase 0 is mask-DMA-gated: I units take a 1-byte fp8 additive mask (half
# the early DMA bytes) and burn idle PE/ACT time instead.
# Phases 1-6: the baseline alternating A/Z/P mix.
# Phase 7 feeds the tail: S units (split DVE+Pool fused STT) evict fast so
# the final AV/finalize chain drains early.
import os

if os.environ.get("K_MID", "base") == "T":
    # P retired: A7/Z4/T3/W2 per phase. ACT ~8.7, DVE ~9.7, Pool ~7.0
    # (norm on Pool) vs cadence 10.3.
    _B0 = "AWZATAZTAWZATZAA"
    _B1 = "ATZAWAZTATZAWZAA"
    _PH7D = "ASZATAZTASZATZAA"
else:
    _B0 = "AZAPAZPAZPAZPZAZ"
    _B1 = "PAZPAZPAZPAZPAZA"
    _PH7D = _B1
_B0 = os.environ.get("K_B0", _B0)
_B1 = os.environ.get("K_B1", _B1)
_PH0 = os.environ.get("K_PH0", _B0)
_PH7 = os.environ.get("K_PH7", os.environ.get("K_B1", _PH7D))
PATTERNS = [_PH0, _B1, _B0, _B1, _B0, _B1, _B0, _PH7]
PATTERNS_SMALL = ["WAZP", "TIZP"]  # ks3 is pure-P -> covers the fp8 P mask
TAIL_FE = int(os.environ.get("K_TAIL_FE", "4"))
AV_START = int(os.environ.get("K_AV_START", "4"))
NORM_POOL = os.environ.get("K_NORM", "dve") == "pool"
# The fused psO-divide finalize races with psO slot reuse under the tile
# scheduler (CoreSim NaN-poisons it); keep the two-op recip+mul finalize.
USE_DIV = os.environ.get("K_DIV", "0") == "1"
N_WARM = int(os.environ.get("K_WARM", "24"))
MS_GPSIMD = os.environ.get("K_MS", "dve") == "gp"
MASK_BIAS = -1280.0  # e5m2-exact; exp((s-1280)/8) == 0 for masked entries


def build_program(s=S, reps=1, patterns=PATTERNS):
    """Build the single-core SPMD program. Returns the compiled Bacc object.

    reps>1 emits the whole body that many times in one NEFF - used to measure
    device time by wall-clock differencing."""
    nc = bacc.Bacc()

    if s < 2048 and patterns is PATTERNS:
        patterns = PATTERNS_SMALL
    KS = s // 128            # number of k strips
    QG = min(1024, s)        # q width of one eviction unit
    NQG = s // QG            # q groups ("halves" at s=2048)
    NCH = QG // 128          # AV q-chunks per group
    CPG = min(4, NCH)        # chunks per psO group

    # Per-head Q^T/K^T with two extra contraction rows (64: a1/b1, 65:
    # a2/b2). Slicing lhsT/rhs to [0:66] adds a1*b1+a2*b2 = B_PRIME to every
    # score of that unit (the Schraudolph bias), at zero matmul cost; [0:64]
    # gives plain scores. Q-side rows are 1.0; K-side rows are 728.0 and
    # -0.25292969 (both bf16-exact, sum 727.74707 vs B_PRIME 727.746979).
    qkT_d = nc.declare_dram_parameter(
        "qkT", [HPC, HD + 2, 2, s], BF16, isOutput=False
    )
    v_d = nc.declare_dram_parameter(
        "v", [s, HPC * (HD + 1)], BF16, isOutput=False
    )
    nmT_d = nc.declare_dram_parameter("nmT", [s, s], BF16, isOutput=False)
    nm8_d = nc.declare_dram_parameter("nm8", [s, s], FP8E5, isOutput=False)
    nm8m_d = nc.declare_dram_parameter("nm8m", [s, s], FP8E4, isOutput=False)
    out_d = nc.declare_dram_parameter("out", [s, HPC * HD], BF16, isOutput=True)

    # Which mask formats each (g, ks) slot needs, from the per-phase paths:
    # A/Z use the bf16 multiplicative mask, P the int16 AND-mask, I the fp8
    # additive mask. Only the needed pieces are DMA'd / kept resident.
    def slot_paths(g, ks):
        return {
            patterns[(g * HPC + h) % len(patterns)][
                ks % len(patterns[(g * HPC + h) % len(patterns)])]
            for h in range(HPC)
        }

    gks = [(g, ks) for g in range(NQG) for ks in range(KS)]
    i_pieces = sorted(t for t in gks if "I" in slot_paths(*t))
    # Strips that are P-path in EVERY phase can take a 1-byte fp8 {1,0}
    # multiplicative mask: the x A' scale folds into the ACT exp bias
    # (ln A'), and Pool's tensor_mul has no dtype-dependent cost. Strips
    # that are Z-path in EVERY phase also take the fp8 mask: their QK slices
    # [0:66] to pre-add B_PRIME, and the STT becomes (psS*A')*m8. Same
    # instructions, same emission order - only the DMA stream shrinks.
    fp8p_pieces = {t for t in gks if slot_paths(*t) == {"P"}}
    fp8z_pieces = {t for t in gks if slot_paths(*t) == {"Z"}}
    fp8m_pieces = fp8p_pieces | fp8z_pieces
    az_pieces = {t for t in gks
                 if (slot_paths(*t) & {"A", "Z", "W", "P", "S", "T"})
                 and t not in fp8m_pieces}

    nm_view = nmT_d[:].rearrange("(ks p) q -> p ks q", p=128)
    nm8_view = nm8_d[:].rearrange("(ks p) q -> p ks q", p=128)
    nm8m_view = nm8m_d[:].rearrange("(ks p) q -> p ks q", p=128)
    v_view = v_d[:].rearrange("(ks p) c -> p ks c", p=128)
    out_view = out_d[:].rearrange("(sq p) c -> p sq c", p=128)

    with tile.TileContext(nc) as tc:
        with (
            tc.tile_pool(name="const", bufs=1) as const,
            tc.tile_pool(name="wq", bufs=1) as wq,
            tc.tile_pool(name="attn", bufs=min(2 * KS + 4, 36)) as apool,
            tc.tile_pool(name="stat", bufs=4) as spool,
            tc.tile_pool(name="oasm", bufs=1) as opool,
            tc.tile_pool(name="psS", bufs=3, space="PSUM") as psS_pool,
            tc.tile_pool(name="psO", bufs=2, space="PSUM") as psO_pool,
        ):
            aux = nc.gpsimd if MS_GPSIMD else nc.vector

            if os.environ.get("K_ZB", "late") == "first":
                zb = const.tile([128, 128], BF16)
                nc.vector.memset(zb, 0.0)

            # Preload the exp table (emitted before any real exp; runs while
            # the first DMAs stream).
            warm = const.tile([128, 1], F32)
            aux.memset(warm, 0.0)
            nc.scalar.activation(warm, warm, mybir.ActivationFunctionType.Exp)

            # fp8e5 identity for the I-path mask-add matmul (tile 1 unused;
            # the [128, 2, 128] shape + memset keep the original const-setup
            # op stream, whose scheduling the rest of the kernel is tuned to).
            identf = const.tile([128, 128], F32)
            make_identity(nc, identf)
            ident8 = const.tile([128, 2, 128], FP8E5)
            aux.memset(ident8, 0.0)
            aux.tensor_copy(out=ident8[:, 0, :], in_=identf)
            # Per-partition bias ln(A') for the I path's exp.
            lnap = const.tile([128, 1], F32)
            aux.memset(lnap, float(np.log(A_PRIME)))

            # Warm the PE clock (cost model p-state ramp) while input DMAs
            # stream: ~3us of dummy matmuls.
            if os.environ.get("K_ZB", "late") == "first":
                pass
            else:
                zb = const.tile([128, 128], BF16)
                nc.vector.memset(zb, 0.0)
            for _ in range(N_WARM):
                wmm = psS_pool.tile([128, QG], F32, tag="psS")
                nc.tensor.matmul(
                    wmm[:, :128], lhsT=zb[0:64, :], rhs=zb[0:64, :],
                    start=True, stop=True,
                )

            def qk_src(h):
                return qkT_d[h]

            def emit_body():
                # Per-head Q^T / K^T: [66, 2, s] (dim1: 0=Q^T, 1=K^T; rows
                # 64-65 are the Schraudolph bias constants).
                qks = []
                for h in range(HPC):
                    qk = wq.tile([HD + 2, 2, s], BF16, tag=f"qkT{h}")
                    qks.append(qk)
                v_sb = wq.tile([128, KS, HPC * (HD + 1)], BF16, tag="vsb")
                nm_sb = wq.tile([128, KS, s], BF16, tag="nm")
                KH = KS // 2
                # All input DMAs ride the SP HWDGE queue (SP has no compute,
                # so ring-full stalls never block a compute sequencer; gpsimd
                # dma_start is SWDGE and would burn Pool engine time). Pieces
                # are ordered by first use; phases run q-group-major, so mask
                # q-group 1 is not needed until ~halfway through the kernel.
                nm8p = {}
                for (g, ks) in i_pieces:
                    t = wq.tile([128, QG], FP8E5, tag=f"nm8_{g}_{ks}",
                                name=f"nm8_{g}_{ks}")
                    nm8p[(g, ks)] = t
                nm8mp = {}
                for (g, ks) in sorted(fp8m_pieces):
                    t = wq.tile([128, QG], FP8E4, tag=f"nm8m_{g}_{ks}",
                                name=f"nm8m_{g}_{ks}")
                    nm8mp[(g, ks)] = t

                def nm_piece(ks, g):
                    if (g, ks) in fp8m_pieces:
                        nc.sync.dma_start(
                            out=nm8mp[(g, ks)],
                            in_=nm8m_view[:, ks, g * QG:(g + 1) * QG],
                        )
                    elif (g, ks) in az_pieces:
                        nc.sync.dma_start(
                            out=nm_sb[:, ks, g * QG:(g + 1) * QG],
                            in_=nm_view[:, ks, g * QG:(g + 1) * QG],
                        )

                def nm8_piece(ks, g):
                    nc.sync.dma_start(
                        out=nm8p[(g, ks)],
                        in_=nm8_view[:, ks, g * QG:(g + 1) * QG],
                    )

                # Heads 0/1 split by channel (same 4-DMA structure as the old
                # by-head pair split) so head 0's slices land first.
                nc.scalar.dma_start(out=qks[0][:, 0, :], in_=qk_src(0)[:, 0, :])
                nc.sync.dma_start(out=qks[0][:, 1, :], in_=qk_src(0)[:, 1, :])
                nc.scalar.dma_start(out=qks[1][:, 0, :], in_=qk_src(1)[:, 0, :])
                nc.sync.dma_start(out=qks[1][:, 1, :], in_=qk_src(1)[:, 1, :])
                # Phase-0 mask pieces in consumption order: I strips need only
                # the 1-byte fp8 piece now (bf16 copy deferred until after v).
                ph0 = patterns[0]
                deferred = []
                for ks in range(KS):
                    if ph0[ks % len(ph0)] == "I":
                        if (0, ks) in i_pieces:
                            nm8_piece(ks, 0)
                        if (0, ks) in az_pieces or (0, ks) in fp8m_pieces:
                            deferred.append(("nm", ks))
                    else:
                        if (0, ks) in az_pieces or (0, ks) in fp8m_pieces:
                            nm_piece(ks, 0)
                        if (0, ks) in i_pieces:
                            deferred.append(("nm8", ks))
                nc.sync.dma_start(out=v_sb[:, :KH], in_=v_view[:, :KH])
                nc.sync.dma_start(out=v_sb[:, KH:], in_=v_view[:, KH:])
                for kind, ks in deferred:
                    (nm_piece if kind == "nm" else nm8_piece)(ks, 0)
                for h in range(2, HPC):
                    nc.sync.dma_start(out=qks[h], in_=qk_src(h))
                for g in range(1, NQG):
                    for ks in range(KS):
                        if (g, ks) in i_pieces:
                            nm8_piece(ks, g)
                        if (g, ks) in az_pieces or (g, ks) in fp8m_pieces:
                            nm_piece(ks, g)


                out_asm = opool.tile([128, KS, HPC * HD], BF16)

                # q-group-major phase order: the first HPC phases only touch
                # mask q-group 0, giving the mask DMA stream headroom.
                phases = [(h, g) for g in range(NQG) for h in range(HPC)]

                def emit_av_chunk(ph, c, av_state):
                    """AV matmuls for q-chunk c of phase ph, plus group
                    finalize (reciprocal + normalize) every CPG chunks.

                    Strips are read in eviction-completion order (Z first,
                    then A, then P): the last strips read are the ones whose
                    masks lag past the phase boundary, so the PE never waits
                    on a straggling Pool/DVE mask with work still in hand."""
                    h, g = ph
                    ats = av_state["ats"]
                    order = av_state["order"]
                    if c % CPG == 0:
                        av_state["psO"] = psO_pool.tile(
                            [128, CPG, 128], F32, tag="psO", name="psO"
                        )
                    psO = av_state["psO"]
                    for i, ks in enumerate(order):
                        _dbg(nc.tensor.matmul(
                            psO[:, c % CPG, 0:HD + 1],
                            lhsT=ats[ks][:, c * 128:(c + 1) * 128],
                            rhs=v_sb[:, ks, h * (HD + 1):(h + 1) * (HD + 1)],
                            start=(i == 0),
                            stop=(i == KS - 1),
                        ), f"AV h{h}g{g} c{c} ks{ks}")
                    fe = av_state.get("fin_every", CPG)
                    if c % fe == fe - 1:
                        fin = (h, g, c - (fe - 1), fe, psO)
                        if c == NCH - 1 and av_state.get("defer_last"):
                            av_state["deferred"] = fin
                        else:
                            emit_finalize(fin, av_state.get("norm_engine"))

                def emit_finalize(fin, norm_engine=None):
                    h, g, c0, fe, psO = fin
                    qc0 = g * NCH + c0
                    eng = norm_engine if norm_engine is not None else (
                        nc.gpsimd if NORM_POOL else nc.vector)
                    if USE_DIV:
                        # Single fused normalize: out = psO[:, :, 0:HD] / Z
                        # (column HD broadcast) - no separate reciprocal.
                        _dbg(eng.tensor_tensor(
                            out_asm[:, qc0:qc0 + fe, h * HD:(h + 1) * HD],
                            psO[:, c0 % CPG:c0 % CPG + fe, 0:HD],
                            psO[:, c0 % CPG:c0 % CPG + fe,
                                HD:HD + 1].to_broadcast([128, fe, HD]),
                            mybir.AluOpType.divide,
                        ), f"norm h{h}g{g} c{c0}")
                        return
                    rec = spool.tile([128, fe], F32, tag="rec", name="rec")
                    _dbg(nc.vector.reciprocal(
                        rec, psO[:, c0 % CPG:c0 % CPG + fe, HD]),
                         f"recip h{h}g{g} c{c0}")
                    _dbg(eng.tensor_mul(
                        out_asm[:, qc0:qc0 + fe, h * HD:(h + 1) * HD],
                        psO[:, c0 % CPG:c0 % CPG + fe, 0:HD],
                        rec.to_broadcast([128, fe, HD]),
                    ), f"norm h{h}g{g} c{c0}")
                    if h == HPC - 1:
                        nc.sync.dma_start(
                            out=out_view[:, qc0:qc0 + fe, :],
                            in_=out_asm[:, qc0:qc0 + fe, :],
                        )

                prev = None  # (phase, {"ats": [...]}) awaiting AV
                pending_fin = []
                unit = 0  # global eviction-unit counter (for path pattern)
                for ph in phases:
                    h, g = ph
                    q0 = g * QG
                    ats = []
                    paths = []
                    for ks in range(KS):
                        # QK for this unit
                        pat = patterns[(unit // KS) % len(patterns)]
                        path = pat[ks % len(pat)]
                        # Pure-Z strips contract over 66 rows: rows 64/65 add
                        # the Schraudolph bias B' to every score.
                        kb = HD + 2 if (g, ks) in fp8z_pieces else HD
                        psS = psS_pool.tile([128, QG], F32, tag="psS")
                        for qc in range(QG // 512):
                            _dbg(nc.tensor.matmul(
                                psS[:, qc * 512:(qc + 1) * 512],
                                lhsT=qks[h][0:kb, 1,
                                            ks * 128:(ks + 1) * 128],
                                rhs=qks[h][0:kb, 0,
                                           q0 + qc * 512:q0 + (qc + 1) * 512],
                                start=True,
                                stop=(path != "I"),
                            ), f"QK h{h}g{g} ks{ks}")
                        if path == "I":
                            # Mask-add on the PE: psS += I^T @ nm8 via a plain
                            # fp8 matmul (~213ns per 512 columns).
                            for qc in range(QG // 512):
                                _dbg(nc.tensor.matmul(
                                    psS[:, qc * 512:(qc + 1) * 512],
                                    lhsT=ident8[:, 0, :],
                                    rhs=nm8p[(g, ks)][:,
                                                      qc * 512:(qc + 1) * 512],
                                    start=False,
                                    stop=True,
                                ), f"maskI h{h}g{g} ks{ks}")
                        # Chunks ride units 4..~12: late enough that the
                        # previous phase's last evictions have drained, early
                        # enough that attn slots recycle before phase p+2.
                        if ks == 1 and pending_fin:
                            emit_finalize(pending_fin.pop())
                        if prev is not None:
                            start = AV_START if KS > 8 else 1
                            den = max(KS - start - 2, 1)
                            for c in range(NCH):
                                if min(start + c * den // NCH, KS - 1) == ks:
                                    emit_av_chunk(prev[0], c, prev[1])
                        # Eviction: psS -> masked bf16 attn tile
                        at = apool.tile([128, QG], BF16, tag="at")
                        nm_slice = nm_sb[:, ks, q0:q0 + QG]
                        unit += 1
                        if path == "I":
                            # Mask already added in PSUM; exp with bias ln(A')
                            # scales the weights to match the other paths.
                            _dbg(nc.scalar.activation(
                                at, psS, mybir.ActivationFunctionType.Exp,
                                scale=0.125, bias=lnap[:],
                            ), f"expI h{h}g{g} ks{ks}")
                        elif path == "Z" and (g, ks) in fp8z_pieces:
                            # Bias pre-added by the QK rows; fp8 {1,0} mask.
                            _dbg(nc.vector.scalar_tensor_tensor(
                                at[:].bitcast(I16),
                                psS[:],
                                A_PRIME,
                                nm8mp[(g, ks)],
                                mybir.AluOpType.mult,
                                mybir.AluOpType.mult,
                            ), f"STT8 h{h}g{g} ks{ks}")
                        elif path == "Z":
                            _dbg(nc.vector.scalar_tensor_tensor(
                                at[:].bitcast(I16),
                                psS[:],
                                B_PRIME,
                                nm_slice,
                                mybir.AluOpType.add,
                                mybir.AluOpType.mult,
                            ), f"STT h{h}g{g} ks{ks}")
                        elif path == "W":
                            _dbg(nc.gpsimd.scalar_tensor_tensor(
                                at[:].bitcast(I16),
                                psS[:],
                                B_PRIME,
                                nm_slice,
                                mybir.AluOpType.add,
                                mybir.AluOpType.mult,
                            ), f"STTW h{h}g{g} ks{ks}")
                        elif path in ("S", "T"):
                            # Split eviction: fast engines take cols [0:HF)
                            # (read by AV chunks 0..NCH/2-1, emitted first);
                            # Pool STT takes [HF:QG).
                            HF = QG // 2
                            if path == "S":
                                _dbg(nc.vector.scalar_tensor_tensor(
                                    at[:, 0:HF].bitcast(I16),
                                    psS[:, 0:HF],
                                    B_PRIME,
                                    nm_slice[:, 0:HF],
                                    mybir.AluOpType.add,
                                    mybir.AluOpType.mult,
                                ), f"STTh h{h}g{g} ks{ks}")
                            else:
                                _dbg(nc.scalar.activation(
                                    at[:, 0:HF], psS[:, 0:HF],
                                    mybir.ActivationFunctionType.Exp,
                                    scale=0.125,
                                ), f"expTh h{h}g{g} ks{ks}")
                                _dbg(nc.vector.tensor_mul(
                                    at[:, 0:HF], at[:, 0:HF],
                                    nm_slice[:, 0:HF]),
                                    f"maskTh h{h}g{g} ks{ks}")
                            _dbg(nc.gpsimd.scalar_tensor_tensor(
                                at[:, HF:QG].bitcast(I16),
                                psS[:, HF:QG],
                                B_PRIME,
                                nm_slice[:, HF:QG],
                                mybir.AluOpType.add,
                                mybir.AluOpType.mult,
                            ), f"STTWh h{h}g{g} ks{ks}")
                        elif path == "A":
                            _dbg(nc.scalar.activation(
                                at, psS, mybir.ActivationFunctionType.Exp,
                                scale=0.125,
                            ), f"expA h{h}g{g} ks{ks}")
                            _dbg(nc.vector.tensor_mul(at, at, nm_slice),
                                 f"maskA h{h}g{g} ks{ks}")
                        elif (g, ks) in fp8p_pieces:
                            # P-path with 1-byte mask: x A' folded into the
                            # exp bias, Pool multiplies by fp8 {1, 0}.
                            _dbg(nc.scalar.activation(
                                at, psS, mybir.ActivationFunctionType.Exp,
                                scale=0.125, bias=lnap[:],
                            ), f"expP8 h{h}g{g} ks{ks}")
                            _dbg(nc.gpsimd.tensor_mul(
                                at, at, nm8mp[(g, ks)]),
                                 f"maskP8 h{h}g{g} ks{ks}")
                        else:
                            _dbg(nc.scalar.activation(
                                at, psS, mybir.ActivationFunctionType.Exp,
                                scale=0.125,
                            ), f"expP h{h}g{g} ks{ks}")
                            _dbg(nc.gpsimd.tensor_mul(at, at, nm_slice),
                                 f"maskP h{h}g{g} ks{ks}")
                        ats.append(at)
                        paths.append(path)
                    rank = {"I": 0, "Z": 1, "S": 2, "T": 3, "A": 4, "W": 5,
                            "P": 6}
                    order = sorted(range(KS), key=lambda k: (rank[paths[k]], k))
                    if prev is not None and prev[1].get("deferred"):
                        pending_fin.append(prev[1]["deferred"])
                    prev = (ph, {"ats": ats, "order": order,
                                 "defer_last": KS > 8})
                # Tail: AV of the final phase. Finalize every 2 chunks (and
                # DMA out per finalize) so the drain chain after the last AV
                # matmul is short.
                prev[1]["defer_last"] = False
                prev[1]["fin_every"] = TAIL_FE
                for fin in pending_fin:
                    emit_finalize(fin)
                for c in range(NCH):
                    emit_av_chunk(prev[0], c, prev[1])

            for _ in range(reps):
                emit_body()
    nc.compile()
    return nc


_CACHE = {}


def _get_nc():
    if "nc" not in _CACHE:
        _CACHE["nc"] = build_program()
    return _CACHE["nc"]


def make_in_maps(q, k, v, mask, s=S):
    """Shard full inputs into 8 per-core input maps (host-side layout prep)."""
    q = np.asarray(q, dtype=np.float32)
    k = np.asarray(k, dtype=np.float32)
    v = np.asarray(v, dtype=np.float32)
    mask = np.asarray(mask)
    nh = q.shape[-1] // HD
    in_maps = []
    for c in range(NCORES):
        b, g = divmod(c, NCORES // B)
        h0 = HPC * g
        qs = q[b].reshape(s, nh, HD)[:, h0:h0 + HPC, :]      # [s, HPC, 64]
        ks_ = k[b].reshape(s, nh, HD)[:, h0:h0 + HPC, :]
        qkT = np.empty((HPC, HD + 2, 2, s), ml_dtypes.bfloat16)
        qkT[:, 0:HD, 0, :] = qs.transpose(1, 2, 0)
        qkT[:, 0:HD, 1, :] = ks_.transpose(1, 2, 0)
        # Schraudolph bias rows: Q-side 1.0; K-side 728.0 and -0.25292969
        # (bf16-exact; their sum ~= B_PRIME).
        qkT[:, HD:HD + 2, 0, :] = 1.0
        qkT[:, HD, 1, :] = 728.0
        qkT[:, HD + 1, 1, :] = -0.25292969
        vh = v[b, :, h0 * HD:(h0 + HPC) * HD].reshape(s, HPC, HD)
        vc = np.concatenate(
            [vh, np.ones((s, HPC, 1), np.float32)], axis=2
        ).reshape(s, HPC * (HD + 1)).astype(ml_dtypes.bfloat16)
        mT = mask[b].T
        nmT = (np.float32(A_PRIME) * (~mT).astype(np.float32)).astype(
            ml_dtypes.bfloat16
        )
        nm8 = (np.float32(MASK_BIAS) * mT.astype(np.float32)).astype(
            ml_dtypes.float8_e5m2
        )
        nm8m = (~mT).astype(np.float32).astype(ml_dtypes.float8_e4m3fn)
        in_maps.append(
            {"qkT": qkT, "v": vc, "nmT": nmT, "nm8": nm8, "nm8m": nm8m}
        )
    return in_maps


def assemble_out(results, s=S, d=D):
    out = np.empty((B, s, d), np.float32)
    for c in range(NCORES):
        b, g = divmod(c, NCORES // B)
        out[b, :, g * HPC * HD:(g + 1) * HPC * HD] = results[c]["out"]
    return out


def kernel(q, k, v, mask):
    from concourse.bass_utils import run_bass_kernel_spmd

    nc = _get_nc()
    in_maps = make_in_maps(q, k, v, mask)
    res = run_bass_kernel_spmd(nc, in_maps, list(range(NCORES))).results
    return assemble_out(res)



# revision 68
# speedup vs baseline: 1.0157x; 1.0027x over previous
# Multi-head attention (B=2, S=2048, D=1024, H=16, head_dim=64) with bool mask,
# sharded across 8 TRN2 NeuronCores: core c -> batch c//4, heads 4*(c%4)..4*(c%4)+3.
#
# Per-core device kernel:
#   scoresT = K @ Q^T                 (PE bf16, [128 k, 1024 q] units)
#   eviction of each psS unit to bf16 attn, split 4 ways to balance engines:
#     'A': ACT exp(scale=1/8) -> DVE mask multiply
#     'P': ACT exp(scale=1/8) -> Pool (gpsimd) mask multiply
#     'Z': one fused DVE scalar_tensor_tensor: i16 <- (psS + B') * m'[k,q],
#          bit-reinterpreted as bf16 == Schraudolph exp(s/8) with the mask
#          folded in. The mask tile holds {A'=23.125, 0}; on the A/P paths the
#          same tile is a plain multiplicative mask whose uniform A' factor
#          cancels in the softmax normalization. B' is tuned so the Z path's
#          mean scale matches the A/P paths' A'*exp(s/8) exactly.
#     'I': additive fp8e5 mask folded into PSUM on the otherwise-idle PE (a
#          DoubleRow identity matmul adds -1280 to masked entries), then a
#          mask-free ACT exp with bias ln(A').
#   AV in direct layout: out[q,d] = attnT^T @ [V|1] per 128-q chunk (PE bf16,
#   full 128 output partitions; column 64 is the softmax denominator Z).
#   normalize: DVE reciprocal + broadcast multiply, assembled in SBUF, DMA out.
#
# Host side (inside kernel()): slice per-core shards, pre-transpose Q/K per head
# ([64, S] head-dim-major, bf16), pre-bake the inverted mask transposed as
# {A', 0} bf16, reassemble the 8 per-core bf16 outputs into [B, S, D] f32.

import sys

import numpy as np

for _p in ("/opt/trn_rl_repo",):
    if _p not in sys.path:
        sys.path.insert(0, _p)

import ml_dtypes

import concourse.bass as bass  # noqa: F401  (engine types reachable via nc)
import concourse.tile as tile
from concourse import bacc, mybir
from concourse.masks import make_identity

F32 = mybir.dt.float32
BF16 = mybir.dt.bfloat16
I16 = mybir.dt.int16
FP8E5 = mybir.dt.float8e5
FP8E4 = mybir.dt.float8e4

S = 2048          # sequence length
HD = 64           # head dim
HPC = 4           # heads per core
NCORES = 8
B = 2
H = 16
D = H * HD

# Schraudolph constants for the Z path. A' is the exact bf16 rounding of
# 128/(8*ln2); B' is tuned (float32, truncating i16 cast) so that
# E[bitcast_bf16(i16((s+B')*A'))] == A' * exp(s/8) over the score distribution.
A_PRIME = 23.125
B_PRIME = 727.746979

# Optional debug map: instruction name -> semantic label (filled when
# DEBUG_LABELS is a dict; costs nothing when None).
DEBUG_LABELS = None


def _dbg(ins, label):
    if DEBUG_LABELS is not None and ins is not None:
        try:
            DEBUG_LABELS[ins.ins.name] = label
        except AttributeError:
            pass

# Per-phase eviction path patterns (16 k-strip units per phase), alternating.
# Single-engine paths: Z = fused DVE bit-trick, W = same on Pool (gpsimd
# STT), A = ACT exp + DVE mask, P = ACT exp + Pool mask (legacy).
# Split paths (two engines, by column halves - frees the psS PSUM slot in
# ~1.0us, under the 3-slot recycle slack of ~1.28us):
#   S = DVE STT on cols [0:H) + Pool STT on [H:QG)
#   T = ACT exp + DVE mask on [0:H) + Pool STT on [H:QG)
# Per-phase cadence is ~10.3us (QK+AV on PE). 6A+4S+6T:
# ACT 9.4us, DVE 8.1us, Pool 9.3us (incl normalize on Pool).
# One pattern per phase (8 phases at s=2048).
# Phase 0 is mask-DMA-gated: I units take a 1-byte fp8 additive mask (half
# the early DMA bytes) and burn idle PE/ACT time instead.
# Phases 1-6: the baseline alternating A/Z/P mix.
# Phase 7 feeds the tail: S units (split DVE+Pool fused STT) evict fast so
# the final AV/finalize chain drains early.
import os

if os.environ.get("K_MID", "base") == "T":
    # P retired: A7/Z4/T3/W2 per phase. ACT ~8.7, DVE ~9.7, Pool ~7.0
    # (norm on Pool) vs cadence 10.3.
    _B0 = "AWZATAZTAWZATZAA"
    _B1 = "ATZAWAZTATZAWZAA"
    _PH7D = "ASZATAZTASZATZAA"
else:
    _B0 = "AZAPAZPAZPAZPZAZ"
    _B1 = "PAZPAZPAZPAZPAZA"
    _PH7D = _B1
_B0 = os.environ.get("K_B0", _B0)
_B1 = os.environ.get("K_B1", _B1)
_PH0 = os.environ.get("K_PH0", _B0)
_PH7 = os.environ.get("K_PH7", os.environ.get("K_B1", _PH7D))
PATTERNS = [_PH0, _B1, _B0, _B1, _B0, _B1, _B0, _PH7]
PATTERNS_SMALL = ["WAZP", "TIZP"]  # ks3 is pure-P -> covers the fp8 P mask
TAIL_FE = int(os.environ.get("K_TAIL_FE", "4"))
AV_START = int(os.environ.get("K_AV_START", "4"))
NORM_POOL = os.environ.get("K_NORM", "dve") == "pool"
# The fused psO-divide finalize races with psO slot reuse under the tile
# scheduler (CoreSim NaN-poisons it); keep the two-op recip+mul finalize.
USE_DIV = os.environ.get("K_DIV", "0") == "1"
N_WARM = int(os.environ.get("K_WARM", "24"))
MS_GPSIMD = os.environ.get("K_MS", "dve") == "gp"
# Defer the head-1 qk DMAs until after this many phase-0 mask pieces
# (0 = emit up front with head 0). Head 1 is first consumed ~10us in;
# position 9 balances earlier mask strips against qk1's own deadline
# (measured: 8 -> 106172, 9 -> 105859, 10 -> 105960, 13 -> 106172).
QK1_POS = int(os.environ.get("K_QK1", "9"))
MASK_BIAS = -1280.0  # e5m2-exact; exp((s-1280)/8) == 0 for masked entries


def build_program(s=S, reps=1, patterns=PATTERNS):
    """Build the single-core SPMD program. Returns the compiled Bacc object.

    reps>1 emits the whole body that many times in one NEFF - used to measure
    device time by wall-clock differencing."""
    nc = bacc.Bacc()

    if s < 2048 and patterns is PATTERNS:
        patterns = PATTERNS_SMALL
    KS = s // 128            # number of k strips
    QG = min(1024, s)        # q width of one eviction unit
    NQG = s // QG            # q groups ("halves" at s=2048)
    NCH = QG // 128          # AV q-chunks per group
    CPG = min(4, NCH)        # chunks per psO group

    # Per-head Q^T/K^T with two extra contraction rows (64: a1/b1, 65:
    # a2/b2). Slicing lhsT/rhs to [0:66] adds a1*b1+a2*b2 = B_PRIME to every
    # score of that unit (the Schraudolph bias), at zero matmul cost; [0:64]
    # gives plain scores. Q-side rows are 1.0; K-side rows are 728.0 and
    # -0.25292969 (both bf16-exact, sum 727.74707 vs B_PRIME 727.746979).
    qkT_d = nc.declare_dram_parameter(
        "qkT", [HPC, HD + 2, 2, s], BF16, isOutput=False
    )
    v_d = nc.declare_dram_parameter(
        "v", [s, HPC * (HD + 1)], BF16, isOutput=False
    )
    nmT_d = nc.declare_dram_parameter("nmT", [s, s], BF16, isOutput=False)
    nm8_d = nc.declare_dram_parameter("nm8", [s, s], FP8E5, isOutput=False)
    nm8m_d = nc.declare_dram_parameter("nm8m", [s, s], FP8E4, isOutput=False)
    out_d = nc.declare_dram_parameter("out", [s, HPC * HD], BF16, isOutput=True)

    # Which mask formats each (g, ks) slot needs, from the per-phase paths:
    # A/Z use the bf16 multiplicative mask, P the int16 AND-mask, I the fp8
    # additive mask. Only the needed pieces are DMA'd / kept resident.
    def slot_paths(g, ks):
        return {
            patterns[(g * HPC + h) % len(patterns)][
                ks % len(patterns[(g * HPC + h) % len(patterns)])]
            for h in range(HPC)
        }

    gks = [(g, ks) for g in range(NQG) for ks in range(KS)]
    i_pieces = sorted(t for t in gks if "I" in slot_paths(*t))
    # Strips that are P-path in EVERY phase can take a 1-byte fp8 {1,0}
    # multiplicative mask: the x A' scale folds into the ACT exp bias
    # (ln A'), and Pool's tensor_mul has no dtype-dependent cost. Strips
    # that are Z-path in EVERY phase also take the fp8 mask: their QK slices
    # [0:66] to pre-add B_PRIME, and the STT becomes (psS*A')*m8. Same
    # instructions, same emission order - only the DMA stream shrinks.
    fp8p_pieces = {t for t in gks if slot_paths(*t) == {"P"}}
    fp8z_pieces = {t for t in gks if slot_paths(*t) == {"Z"}}
    fp8m_pieces = fp8p_pieces | fp8z_pieces
    az_pieces = {t for t in gks
                 if (slot_paths(*t) & {"A", "Z", "W", "P", "S", "T"})
                 and t not in fp8m_pieces}

    nm_view = nmT_d[:].rearrange("(ks p) q -> p ks q", p=128)
    nm8_view = nm8_d[:].rearrange("(ks p) q -> p ks q", p=128)
    nm8m_view = nm8m_d[:].rearrange("(ks p) q -> p ks q", p=128)
    v_view = v_d[:].rearrange("(ks p) c -> p ks c", p=128)
    out_view = out_d[:].rearrange("(sq p) c -> p sq c", p=128)

    with tile.TileContext(nc) as tc:
        with (
            tc.tile_pool(name="const", bufs=1) as const,
            tc.tile_pool(name="wq", bufs=1) as wq,
            tc.tile_pool(name="attn", bufs=min(2 * KS + 4, 36)) as apool,
            tc.tile_pool(name="stat", bufs=4) as spool,
            tc.tile_pool(name="oasm", bufs=1) as opool,
            tc.tile_pool(name="psS", bufs=3, space="PSUM") as psS_pool,
            tc.tile_pool(name="psO", bufs=2, space="PSUM") as psO_pool,
        ):
            aux = nc.gpsimd if MS_GPSIMD else nc.vector

            if os.environ.get("K_ZB", "late") == "first":
                zb = const.tile([128, 128], BF16)
                nc.vector.memset(zb, 0.0)

            # Preload the exp table (emitted before any real exp; runs while
            # the first DMAs stream).
            warm = const.tile([128, 1], F32)
            aux.memset(warm, 0.0)
            nc.scalar.activation(warm, warm, mybir.ActivationFunctionType.Exp)

            # fp8e5 identity for the I-path mask-add matmul (tile 1 unused;
            # the [128, 2, 128] shape + memset keep the original const-setup
            # op stream, whose scheduling the rest of the kernel is tuned to).
            identf = const.tile([128, 128], F32)
            make_identity(nc, identf)
            ident8 = const.tile([128, 2, 128], FP8E5)
            aux.memset(ident8, 0.0)
            aux.tensor_copy(out=ident8[:, 0, :], in_=identf)
            # Per-partition bias ln(A') for the I path's exp.
            lnap = const.tile([128, 1], F32)
            aux.memset(lnap, float(np.log(A_PRIME)))

            # Warm the PE clock (cost model p-state ramp) while input DMAs
            # stream: ~3us of dummy matmuls.
            if os.environ.get("K_ZB", "late") == "first":
                pass
            else:
                zb = const.tile([128, 128], BF16)
                nc.vector.memset(zb, 0.0)
            for _ in range(N_WARM):
                wmm = psS_pool.tile([128, QG], F32, tag="psS")
                nc.tensor.matmul(
                    wmm[:, :128], lhsT=zb[0:64, :], rhs=zb[0:64, :],
                    start=True, stop=True,
                )

            def qk_src(h):
                return qkT_d[h]

            def emit_body():
                # Per-head Q^T / K^T: [66, 2, s] (dim1: 0=Q^T, 1=K^T; rows
                # 64-65 are the Schraudolph bias constants).
                qks = []
                for h in range(HPC):
                    qk = wq.tile([HD + 2, 2, s], BF16, tag=f"qkT{h}")
                    qks.append(qk)
                v_sb = wq.tile([128, KS, HPC * (HD + 1)], BF16, tag="vsb")
                nm_sb = wq.tile([128, KS, s], BF16, tag="nm")
                KH = KS // 2
                # All input DMAs ride the SP HWDGE queue (SP has no compute,
                # so ring-full stalls never block a compute sequencer; gpsimd
                # dma_start is SWDGE and would burn Pool engine time). Pieces
                # are ordered by first use; phases run q-group-major, so mask
                # q-group 1 is not needed until ~halfway through the kernel.
                nm8p = {}
                for (g, ks) in i_pieces:
                    t = wq.tile([128, QG], FP8E5, tag=f"nm8_{g}_{ks}",
                                name=f"nm8_{g}_{ks}")
                    nm8p[(g, ks)] = t
                nm8mp = {}
                for (g, ks) in sorted(fp8m_pieces):
                    t = wq.tile([128, QG], FP8E4, tag=f"nm8m_{g}_{ks}",
                                name=f"nm8m_{g}_{ks}")
                    nm8mp[(g, ks)] = t

                def nm_piece(ks, g):
                    if (g, ks) in fp8m_pieces:
                        nc.sync.dma_start(
                            out=nm8mp[(g, ks)],
                            in_=nm8m_view[:, ks, g * QG:(g + 1) * QG],
                        )
                    elif (g, ks) in az_pieces:
                        nc.sync.dma_start(
                            out=nm_sb[:, ks, g * QG:(g + 1) * QG],
                            in_=nm_view[:, ks, g * QG:(g + 1) * QG],
                        )

                def nm8_piece(ks, g):
                    nc.sync.dma_start(
                        out=nm8p[(g, ks)],
                        in_=nm8_view[:, ks, g * QG:(g + 1) * QG],
                    )

                # Heads 0/1 split by channel so head 0's slices land first.
                # Head 1 is not consumed until phase 1 (~10us in), so its two
                # DMAs are deferred into the phase-0 mask stream (after piece
                # QK1_POS) - every earlier mask strip arrives ~1.6us sooner.
                def qk1_dmas():
                    nc.scalar.dma_start(
                        out=qks[1][:, 0, :], in_=qk_src(1)[:, 0, :]
                    )
                    nc.sync.dma_start(
                        out=qks[1][:, 1, :], in_=qk_src(1)[:, 1, :]
                    )

                nc.scalar.dma_start(out=qks[0][:, 0, :], in_=qk_src(0)[:, 0, :])
                nc.sync.dma_start(out=qks[0][:, 1, :], in_=qk_src(0)[:, 1, :])
                if QK1_POS == 0 or KS <= 8:
                    qk1_dmas()
                # Phase-0 mask pieces in consumption order: I strips need only
                # the 1-byte fp8 piece now (bf16 copy deferred until after v).
                ph0 = patterns[0]
                deferred = []
                for ks in range(KS):
                    if QK1_POS != 0 and KS > 8 and ks == QK1_POS:
                        qk1_dmas()
                    if ph0[ks % len(ph0)] == "I":
                        if (0, ks) in i_pieces:
                            nm8_piece(ks, 0)
                        if (0, ks) in az_pieces or (0, ks) in fp8m_pieces:
                            deferred.append(("nm", ks))
                    else:
                        if (0, ks) in az_pieces or (0, ks) in fp8m_pieces:
                            nm_piece(ks, 0)
                        if (0, ks) in i_pieces:
                            deferred.append(("nm8", ks))
                nc.sync.dma_start(out=v_sb[:, :KH], in_=v_view[:, :KH])
                nc.sync.dma_start(out=v_sb[:, KH:], in_=v_view[:, KH:])
                for kind, ks in deferred:
                    (nm_piece if kind == "nm" else nm8_piece)(ks, 0)
                for h in range(2, HPC):
                    nc.sync.dma_start(out=qks[h], in_=qk_src(h))
                for g in range(1, NQG):
                    for ks in range(KS):
                        if (g, ks) in i_pieces:
                            nm8_piece(ks, g)
                        if (g, ks) in az_pieces or (g, ks) in fp8m_pieces:
                            nm_piece(ks, g)


                out_asm = opool.tile([128, KS, HPC * HD], BF16)

                # q-group-major phase order: the first HPC phases only touch
                # mask q-group 0, giving the mask DMA stream headroom.
                phases = [(h, g) for g in range(NQG) for h in range(HPC)]

                def emit_av_chunk(ph, c, av_state):
                    """AV matmuls for q-chunk c of phase ph, plus group
                    finalize (reciprocal + normalize) every CPG chunks.

                    Strips are read in eviction-completion order (Z first,
                    then A, then P): the last strips read are the ones whose
                    masks lag past the phase boundary, so the PE never waits
                    on a straggling Pool/DVE mask with work still in hand."""
                    h, g = ph
                    ats = av_state["ats"]
                    order = av_state["order"]
                    if c % CPG == 0:
                        av_state["psO"] = psO_pool.tile(
                            [128, CPG, 128], F32, tag="psO", name="psO"
                        )
                    psO = av_state["psO"]
                    for i, ks in enumerate(order):
                        _dbg(nc.tensor.matmul(
                            psO[:, c % CPG, 0:HD + 1],
                            lhsT=ats[ks][:, c * 128:(c + 1) * 128],
                            rhs=v_sb[:, ks, h * (HD + 1):(h + 1) * (HD + 1)],
                            start=(i == 0),
                            stop=(i == KS - 1),
                        ), f"AV h{h}g{g} c{c} ks{ks}")
                    fe = av_state.get("fin_every", CPG)
                    if c % fe == fe - 1:
                        fin = (h, g, c - (fe - 1), fe, psO)
                        if c == NCH - 1 and av_state.get("defer_last"):
                            av_state["deferred"] = fin
                        else:
                            emit_finalize(fin, av_state.get("norm_engine"))

                def emit_finalize(fin, norm_engine=None):
                    h, g, c0, fe, psO = fin
                    qc0 = g * NCH + c0
                    eng = norm_engine if norm_engine is not None else (
                        nc.gpsimd if NORM_POOL else nc.vector)
                    if USE_DIV:
                        # Single fused normalize: out = psO[:, :, 0:HD] / Z
                        # (column HD broadcast) - no separate reciprocal.
                        _dbg(eng.tensor_tensor(
                            out_asm[:, qc0:qc0 + fe, h * HD:(h + 1) * HD],
                            psO[:, c0 % CPG:c0 % CPG + fe, 0:HD],
                            psO[:, c0 % CPG:c0 % CPG + fe,
                                HD:HD + 1].to_broadcast([128, fe, HD]),
                            mybir.AluOpType.divide,
                        ), f"norm h{h}g{g} c{c0}")
                        return
                    rec = spool.tile([128, fe], F32, tag="rec", name="rec")
                    _dbg(nc.vector.reciprocal(
                        rec, psO[:, c0 % CPG:c0 % CPG + fe, HD]),
                         f"recip h{h}g{g} c{c0}")
                    _dbg(eng.tensor_mul(
                        out_asm[:, qc0:qc0 + fe, h * HD:(h + 1) * HD],
                        psO[:, c0 % CPG:c0 % CPG + fe, 0:HD],
                        rec.to_broadcast([128, fe, HD]),
                    ), f"norm h{h}g{g} c{c0}")
                    if h == HPC - 1:
                        nc.sync.dma_start(
                            out=out_view[:, qc0:qc0 + fe, :],
                            in_=out_asm[:, qc0:qc0 + fe, :],
                        )

                prev = None  # (phase, {"ats": [...]}) awaiting AV
                pending_fin = []
                unit = 0  # global eviction-unit counter (for path pattern)
                for ph in phases:
                    h, g = ph
                    q0 = g * QG
                    ats = []
                    paths = []
                    for ks in range(KS):
                        # QK for this unit
                        pat = patterns[(unit // KS) % len(patterns)]
                        path = pat[ks % len(pat)]
                        # Pure-Z strips contract over 66 rows: rows 64/65 add
                        # the Schraudolph bias B' to every score.
                        kb = HD + 2 if (g, ks) in fp8z_pieces else HD
                        psS = psS_pool.tile([128, QG], F32, tag="psS")
                        for qc in range(QG // 512):
                            _dbg(nc.tensor.matmul(
                                psS[:, qc * 512:(qc + 1) * 512],
                                lhsT=qks[h][0:kb, 1,
                                            ks * 128:(ks + 1) * 128],
                                rhs=qks[h][0:kb, 0,
                                           q0 + qc * 512:q0 + (qc + 1) * 512],
                                start=True,
                                stop=(path != "I"),
                            ), f"QK h{h}g{g} ks{ks}")
                        if path == "I":
                            # Mask-add on the PE: psS += I^T @ nm8 via a plain
                            # fp8 matmul (~213ns per 512 columns).
                            for qc in range(QG // 512):
                                _dbg(nc.tensor.matmul(
                                    psS[:, qc * 512:(qc + 1) * 512],
                                    lhsT=ident8[:, 0, :],
                                    rhs=nm8p[(g, ks)][:,
                                                      qc * 512:(qc + 1) * 512],
                                    start=False,
                                    stop=True,
                                ), f"maskI h{h}g{g} ks{ks}")
                        # Chunks ride units 4..~12: late enough that the
                        # previous phase's last evictions have drained, early
                        # enough that attn slots recycle before phase p+2.
                        if ks == 1 and pending_fin:
                            emit_finalize(pending_fin.pop())
                        if prev is not None:
                            start = AV_START if KS > 8 else 1
                            den = max(KS - start - 2, 1)
                            for c in range(NCH):
                                if min(start + c * den // NCH, KS - 1) == ks:
                                    emit_av_chunk(prev[0], c, prev[1])
                        # Eviction: psS -> masked bf16 attn tile
                        at = apool.tile([128, QG], BF16, tag="at")
                        nm_slice = nm_sb[:, ks, q0:q0 + QG]
                        unit += 1
                        if path == "I":
                            # Mask already added in PSUM; exp with bias ln(A')
                            # scales the weights to match the other paths.
                            _dbg(nc.scalar.activation(
                                at, psS, mybir.ActivationFunctionType.Exp,
                                scale=0.125, bias=lnap[:],
                            ), f"expI h{h}g{g} ks{ks}")
                        elif path == "Z" and (g, ks) in fp8z_pieces:
                            # Bias pre-added by the QK rows; fp8 {1,0} mask.
                            _dbg(nc.vector.scalar_tensor_tensor(
                                at[:].bitcast(I16),
                                psS[:],
                                A_PRIME,
                                nm8mp[(g, ks)],
                                mybir.AluOpType.mult,
                                mybir.AluOpType.mult,
                            ), f"STT8 h{h}g{g} ks{ks}")
                        elif path == "Z":
                            _dbg(nc.vector.scalar_tensor_tensor(
                                at[:].bitcast(I16),
                                psS[:],
                                B_PRIME,
                                nm_slice,
                                mybir.AluOpType.add,
                                mybir.AluOpType.mult,
                            ), f"STT h{h}g{g} ks{ks}")
                        elif path == "W":
                            _dbg(nc.gpsimd.scalar_tensor_tensor(
                                at[:].bitcast(I16),
                                psS[:],
                                B_PRIME,
                                nm_slice,
                                mybir.AluOpType.add,
                                mybir.AluOpType.mult,
                            ), f"STTW h{h}g{g} ks{ks}")
                        elif path in ("S", "T"):
                            # Split eviction: fast engines take cols [0:HF)
                            # (read by AV chunks 0..NCH/2-1, emitted first);
                            # Pool STT takes [HF:QG).
                            HF = QG // 2
                            if path == "S":
                                _dbg(nc.vector.scalar_tensor_tensor(
                                    at[:, 0:HF].bitcast(I16),
                                    psS[:, 0:HF],
                                    B_PRIME,
                                    nm_slice[:, 0:HF],
                                    mybir.AluOpType.add,
                                    mybir.AluOpType.mult,
                                ), f"STTh h{h}g{g} ks{ks}")
                            else:
                                _dbg(nc.scalar.activation(
                                    at[:, 0:HF], psS[:, 0:HF],
                                    mybir.ActivationFunctionType.Exp,
                                    scale=0.125,
                                ), f"expTh h{h}g{g} ks{ks}")
                                _dbg(nc.vector.tensor_mul(
                                    at[:, 0:HF], at[:, 0:HF],
                                    nm_slice[:, 0:HF]),
                                    f"maskTh h{h}g{g} ks{ks}")
                            _dbg(nc.gpsimd.scalar_tensor_tensor(
                                at[:, HF:QG].bitcast(I16),
                                psS[:, HF:QG],
                                B_PRIME,
                                nm_slice[:, HF:QG],
                                mybir.AluOpType.add,
                                mybir.AluOpType.mult,
                            ), f"STTWh h{h}g{g} ks{ks}")
                        elif path == "A":
                            _dbg(nc.scalar.activation(
                                at, psS, mybir.ActivationFunctionType.Exp,
                                scale=0.125,
                            ), f"expA h{h}g{g} ks{ks}")
                            _dbg(nc.vector.tensor_mul(at, at, nm_slice),
                                 f"maskA h{h}g{g} ks{ks}")
                        elif (g, ks) in fp8p_pieces:
                            # P-path with 1-byte mask: x A' folded into the
                            # exp bias, Pool multiplies by fp8 {1, 0}.
                            _dbg(nc.scalar.activation(
                                at, psS, mybir.ActivationFunctionType.Exp,
                                scale=0.125, bias=lnap[:],
                            ), f"expP8 h{h}g{g} ks{ks}")
                            _dbg(nc.gpsimd.tensor_mul(
                                at, at, nm8mp[(g, ks)]),
                                 f"maskP8 h{h}g{g} ks{ks}")
                        else:
                            _dbg(nc.scalar.activation(
                                at, psS, mybir.ActivationFunctionType.Exp,
                                scale=0.125,
                            ), f"expP h{h}g{g} ks{ks}")
                            _dbg(nc.gpsimd.tensor_mul(at, at, nm_slice),
                                 f"maskP h{h}g{g} ks{ks}")
                        ats.append(at)
                        paths.append(path)
                    rank = {"I": 0, "Z": 1, "S": 2, "T": 3, "A": 4, "W": 5,
                            "P": 6}
                    order = sorted(range(KS), key=lambda k: (rank[paths[k]], k))
                    if prev is not None and prev[1].get("deferred"):
                        pending_fin.append(prev[1]["deferred"])
                    prev = (ph, {"ats": ats, "order": order,
                                 "defer_last": KS > 8})
                # Tail: AV of the final phase. Finalize every 2 chunks (and
                # DMA out per finalize) so the drain chain after the last AV
                # matmul is short.
                prev[1]["defer_last"] = False
                prev[1]["fin_every"] = TAIL_FE
                for fin in pending_fin:
                    emit_finalize(fin)
                for c in range(NCH):
                    emit_av_chunk(prev[0], c, prev[1])

            for _ in range(reps):
                emit_body()
    nc.compile()
    return nc


_CACHE = {}


def _get_nc():
    if "nc" not in _CACHE:
        _CACHE["nc"] = build_program()
    return _CACHE["nc"]


def make_in_maps(q, k, v, mask, s=S):
    """Shard full inputs into 8 per-core input maps (host-side layout prep)."""
    q = np.asarray(q, dtype=np.float32)
    k = np.asarray(k, dtype=np.float32)
    v = np.asarray(v, dtype=np.float32)
    mask = np.asarray(mask)
    nh = q.shape[-1] // HD
    in_maps = []
    for c in range(NCORES):
        b, g = divmod(c, NCORES // B)
        h0 = HPC * g
        qs = q[b].reshape(s, nh, HD)[:, h0:h0 + HPC, :]      # [s, HPC, 64]
        ks_ = k[b].reshape(s, nh, HD)[:, h0:h0 + HPC, :]
        qkT = np.empty((HPC, HD + 2, 2, s), ml_dtypes.bfloat16)
        qkT[:, 0:HD, 0, :] = qs.transpose(1, 2, 0)
        qkT[:, 0:HD, 1, :] = ks_.transpose(1, 2, 0)
        # Schraudolph bias rows: Q-side 1.0; K-side 728.0 and -0.25292969
        # (bf16-exact; their sum ~= B_PRIME).
        qkT[:, HD:HD + 2, 0, :] = 1.0
        qkT[:, HD, 1, :] = 728.0
        qkT[:, HD + 1, 1, :] = -0.25292969
        vh = v[b, :, h0 * HD:(h0 + HPC) * HD].reshape(s, HPC, HD)
        vc = np.concatenate(
            [vh, np.ones((s, HPC, 1), np.float32)], axis=2
        ).reshape(s, HPC * (HD + 1)).astype(ml_dtypes.bfloat16)
        mT = mask[b].T
        nmT = (np.float32(A_PRIME) * (~mT).astype(np.float32)).astype(
            ml_dtypes.bfloat16
        )
        nm8 = (np.float32(MASK_BIAS) * mT.astype(np.float32)).astype(
            ml_dtypes.float8_e5m2
        )
        nm8m = (~mT).astype(np.float32).astype(ml_dtypes.float8_e4m3fn)
        in_maps.append(
            {"qkT": qkT, "v": vc, "nmT": nmT, "nm8": nm8, "nm8m": nm8m}
        )
    return in_maps


def assemble_out(results, s=S, d=D):
    out = np.empty((B, s, d), np.float32)
    for c in range(NCORES):
        b, g = divmod(c, NCORES // B)
        out[b, :, g * HPC * HD:(g + 1) * HPC * HD] = results[c]["out"]
    return out


def kernel(q, k, v, mask):
    from concourse.bass_utils import run_bass_kernel_spmd

    nc = _get_nc()
    in_maps = make_in_maps(q, k, v, mask)
    res = run_bass_kernel_spmd(nc, in_maps, list(range(NCORES))).results
    return assemble_out(res)



# revision 77
# speedup vs baseline: 1.0161x; 1.0005x over previous
# Multi-head attention (B=2, S=2048, D=1024, H=16, head_dim=64) with bool mask,
# sharded across 8 TRN2 NeuronCores: core c -> batch c//4, heads 4*(c%4)..4*(c%4)+3.
#
# Per-core device kernel:
#   scoresT = K @ Q^T                 (PE bf16, [128 k, 1024 q] units)
#   eviction of each psS unit to bf16 attn, split 4 ways to balance engines:
#     'A': ACT exp(scale=1/8) -> DVE mask multiply
#     'P': ACT exp(scale=1/8) -> Pool (gpsimd) mask multiply
#     'Z': one fused DVE scalar_tensor_tensor: i16 <- (psS + B') * m'[k,q],
#          bit-reinterpreted as bf16 == Schraudolph exp(s/8) with the mask
#          folded in. The mask tile holds {A'=23.125, 0}; on the A/P paths the
#          same tile is a plain multiplicative mask whose uniform A' factor
#          cancels in the softmax normalization. B' is tuned so the Z path's
#          mean scale matches the A/P paths' A'*exp(s/8) exactly.
#     'I': additive fp8e5 mask folded into PSUM on the otherwise-idle PE (a
#          DoubleRow identity matmul adds -1280 to masked entries), then a
#          mask-free ACT exp with bias ln(A').
#   AV in direct layout: out[q,d] = attnT^T @ [V|1] per 128-q chunk (PE bf16,
#   full 128 output partitions; column 64 is the softmax denominator Z).
#   normalize: DVE reciprocal + broadcast multiply, assembled in SBUF, DMA out.
#
# Host side (inside kernel()): slice per-core shards, pre-transpose Q/K per head
# ([64, S] head-dim-major, bf16), pre-bake the inverted mask transposed as
# {A', 0} bf16, reassemble the 8 per-core bf16 outputs into [B, S, D] f32.

import sys

import numpy as np

for _p in ("/opt/trn_rl_repo",):
    if _p not in sys.path:
        sys.path.insert(0, _p)

import ml_dtypes

import concourse.bass as bass  # noqa: F401  (engine types reachable via nc)
import concourse.tile as tile
from concourse import bacc, mybir
from concourse.masks import make_identity

F32 = mybir.dt.float32
BF16 = mybir.dt.bfloat16
I16 = mybir.dt.int16
FP8E5 = mybir.dt.float8e5
FP8E4 = mybir.dt.float8e4

S = 2048          # sequence length
HD = 64           # head dim
HPC = 4           # heads per core
NCORES = 8
B = 2
H = 16
D = H * HD

# Schraudolph constants for the Z path. A' is the exact bf16 rounding of
# 128/(8*ln2); B' is tuned (float32, truncating i16 cast) so that
# E[bitcast_bf16(i16((s+B')*A'))] == A' * exp(s/8) over the score distribution.
A_PRIME = 23.125
B_PRIME = 727.746979

# Optional debug map: instruction name -> semantic label (filled when
# DEBUG_LABELS is a dict; costs nothing when None).
DEBUG_LABELS = None


def _dbg(ins, label):
    if DEBUG_LABELS is not None and ins is not None:
        try:
            DEBUG_LABELS[ins.ins.name] = label
        except AttributeError:
            pass

# Per-phase eviction path patterns (16 k-strip units per phase), alternating.
# Single-engine paths: Z = fused DVE bit-trick, W = same on Pool (gpsimd
# STT), A = ACT exp + DVE mask, P = ACT exp + Pool mask (legacy).
# Split paths (two engines, by column halves - frees the psS PSUM slot in
# ~1.0us, under the 3-slot recycle slack of ~1.28us):
#   S = DVE STT on cols [0:H) + Pool STT on [H:QG)
#   T = ACT exp + DVE mask on [0:H) + Pool STT on [H:QG)
# Per-phase cadence is ~10.3us (QK+AV on PE). 6A+4S+6T:
# ACT 9.4us, DVE 8.1us, Pool 9.3us (incl normalize on Pool).
# One pattern per phase (8 phases at s=2048).
# Phase 0 is mask-DMA-gated: I units take a 1-byte fp8 additive mask (half
# the early DMA bytes) and burn idle PE/ACT time instead.
# Phases 1-6: the baseline alternating A/Z/P mix.
# Phase 7 feeds the tail: S units (split DVE+Pool fused STT) evict fast so
# the final AV/finalize chain drains early.
import os

if os.environ.get("K_MID", "base") == "T":
    # P retired: A7/Z4/T3/W2 per phase. ACT ~8.7, DVE ~9.7, Pool ~7.0
    # (norm on Pool) vs cadence 10.3.
    _B0 = "AWZATAZTAWZATZAA"
    _B1 = "ATZAWAZTATZAWZAA"
    _PH7D = "ASZATAZTASZATZAA"
else:
    _B0 = "AZAPAZPAZPAZPZAZ"
    _B1 = "PAZPAZPAZPAZPAZA"
    _PH7D = _B1
_B0 = os.environ.get("K_B0", _B0)
_B1 = os.environ.get("K_B1", _B1)
_PH0 = os.environ.get("K_PH0", _B0)
_PH7 = os.environ.get("K_PH7", os.environ.get("K_B1", _PH7D))
PATTERNS = [_PH0, _B1, _B0, _B1, _B0, _B1, _B0, _PH7]
PATTERNS_SMALL = ["WAZP", "TIZP"]  # ks3 is pure-P -> covers the fp8 P mask
TAIL_FE = int(os.environ.get("K_TAIL_FE", "4"))
AV_START = int(os.environ.get("K_AV_START", "4"))
NORM_POOL = os.environ.get("K_NORM", "dve") == "pool"
# The fused psO-divide finalize races with psO slot reuse under the tile
# scheduler (CoreSim NaN-poisons it); keep the two-op recip+mul finalize.
USE_DIV = os.environ.get("K_DIV", "0") == "1"
N_WARM = int(os.environ.get("K_WARM", "24"))
MS_GPSIMD = os.environ.get("K_MS", "dve") == "gp"
# Defer the head-1 qk DMAs until after this many phase-0 mask pieces
# (0 = emit up front with head 0). Head 1 is first consumed ~10us in;
# position 9 balances earlier mask strips against qk1's own deadline
# (measured: 8 -> 106172, 9 -> 105859, 10 -> 105960, 13 -> 106172).
QK1_POS = int(os.environ.get("K_QK1", "9"))
ZX_ON = os.environ.get("K_ZX", "1") == "1"
MASK_BIAS = -1280.0  # e5m2-exact; exp((s-1280)/8) == 0 for masked entries


def build_program(s=S, reps=1, patterns=PATTERNS):
    """Build the single-core SPMD program. Returns the compiled Bacc object.

    reps>1 emits the whole body that many times in one NEFF - used to measure
    device time by wall-clock differencing."""
    nc = bacc.Bacc()

    if s < 2048 and patterns is PATTERNS:
        patterns = PATTERNS_SMALL
    KS = s // 128            # number of k strips
    QG = min(1024, s)        # q width of one eviction unit
    NQG = s // QG            # q groups ("halves" at s=2048)
    NCH = QG // 128          # AV q-chunks per group
    CPG = min(4, NCH)        # chunks per psO group

    # Per-head Q^T/K^T with two extra contraction rows (64: a1/b1, 65:
    # a2/b2). Slicing lhsT/rhs to [0:66] adds a1*b1+a2*b2 = B_PRIME to every
    # score of that unit (the Schraudolph bias), at zero matmul cost; [0:64]
    # gives plain scores. Q-side rows are 1.0; K-side rows are 728.0 and
    # -0.25292969 (both bf16-exact, sum 727.74707 vs B_PRIME 727.746979).
    qkT_d = nc.declare_dram_parameter(
        "qkT", [HPC, HD + 2, 2, s], BF16, isOutput=False
    )
    v_d = nc.declare_dram_parameter(
        "v", [s, HPC * (HD + 1)], BF16, isOutput=False
    )
    nmT_d = nc.declare_dram_parameter("nmT", [s, s], BF16, isOutput=False)
    nm8_d = nc.declare_dram_parameter("nm8", [s, s], FP8E5, isOutput=False)
    nm8m_d = nc.declare_dram_parameter("nm8m", [s, s], FP8E4, isOutput=False)
    out_d = nc.declare_dram_parameter("out", [s, HPC * HD], BF16, isOutput=True)

    # Which mask formats each (g, ks) slot needs, from the per-phase paths:
    # A/Z use the bf16 multiplicative mask, P the int16 AND-mask, I the fp8
    # additive mask. Only the needed pieces are DMA'd / kept resident.
    def slot_paths(g, ks):
        return {
            patterns[(g * HPC + h) % len(patterns)][
                ks % len(patterns[(g * HPC + h) % len(patterns)])]
            for h in range(HPC)
        }

    gks = [(g, ks) for g in range(NQG) for ks in range(KS)]
    i_pieces = sorted(t for t in gks if "I" in slot_paths(*t))
    # Strips that are P-path in EVERY phase can take a 1-byte fp8 {1,0}
    # multiplicative mask: the x A' scale folds into the ACT exp bias
    # (ln A'), and Pool's tensor_mul has no dtype-dependent cost. Strips
    # that are Z-path in EVERY phase also take the fp8 mask: their QK slices
    # [0:66] to pre-add B_PRIME, and the STT becomes (psS*A')*m8. Same
    # instructions, same emission order - only the DMA stream shrinks.
    fp8p_pieces = {t for t in gks if slot_paths(*t) == {"P"}}
    fp8z_pieces = {t for t in gks if slot_paths(*t) == {"Z"}}
    # Mixed Z/A strips of q-group 0 whose Z phases come first (B0): give
    # them an EARLY fp8 copy for the Z units (bias-in-QK) and defer the
    # bf16 copy (only phase-1/3 A units need it, ~11-17us in) past the
    # DMA-critical window. +0.375MB total DMA, -1.1us of early delivery.
    if ZX_ON and KS > 8 and all(
        patterns[i][k] == ("Z" if i % 2 == 0 else "A")
        for k in (1, 13, 15) for i in range(4)
    ):
        fp8zx_pieces = {(0, 1), (0, 13), (0, 15)}
    else:
        fp8zx_pieces = set()
    fp8z_all = fp8z_pieces | fp8zx_pieces
    fp8m_pieces = fp8p_pieces | fp8z_pieces
    az_pieces = {t for t in gks
                 if (slot_paths(*t) & {"A", "Z", "W", "P", "S", "T"})
                 and t not in fp8m_pieces}

    nm_view = nmT_d[:].rearrange("(ks p) q -> p ks q", p=128)
    nm8_view = nm8_d[:].rearrange("(ks p) q -> p ks q", p=128)
    nm8m_view = nm8m_d[:].rearrange("(ks p) q -> p ks q", p=128)
    v_view = v_d[:].rearrange("(ks p) c -> p ks c", p=128)
    out_view = out_d[:].rearrange("(sq p) c -> p sq c", p=128)

    with tile.TileContext(nc) as tc:
        with (
            tc.tile_pool(name="const", bufs=1) as const,
            tc.tile_pool(name="wq", bufs=1) as wq,
            tc.tile_pool(name="attn", bufs=min(2 * KS + 4, 36)) as apool,
            tc.tile_pool(name="stat", bufs=4) as spool,
            tc.tile_pool(name="oasm", bufs=1) as opool,
            tc.tile_pool(name="psS", bufs=3, space="PSUM") as psS_pool,
            tc.tile_pool(name="psO", bufs=2, space="PSUM") as psO_pool,
        ):
            aux = nc.gpsimd if MS_GPSIMD else nc.vector

            if os.environ.get("K_ZB", "late") == "first":
                zb = const.tile([128, 128], BF16)
                nc.vector.memset(zb, 0.0)

            # Preload the exp table (emitted before any real exp; runs while
            # the first DMAs stream).
            warm = const.tile([128, 1], F32)
            aux.memset(warm, 0.0)
            nc.scalar.activation(warm, warm, mybir.ActivationFunctionType.Exp)

            # fp8e5 identity for the I-path mask-add matmul (tile 1 unused;
            # the [128, 2, 128] shape + memset keep the original const-setup
            # op stream, whose scheduling the rest of the kernel is tuned to).
            identf = const.tile([128, 128], F32)
            make_identity(nc, identf)
            ident8 = const.tile([128, 2, 128], FP8E5)
            aux.memset(ident8, 0.0)
            aux.tensor_copy(out=ident8[:, 0, :], in_=identf)
            # Per-partition bias ln(A') for the I path's exp.
            lnap = const.tile([128, 1], F32)
            aux.memset(lnap, float(np.log(A_PRIME)))

            # Warm the PE clock (cost model p-state ramp) while input DMAs
            # stream: ~3us of dummy matmuls.
            if os.environ.get("K_ZB", "late") == "first":
                pass
            else:
                zb = const.tile([128, 128], BF16)
                nc.vector.memset(zb, 0.0)
            for _ in range(N_WARM):
                wmm = psS_pool.tile([128, QG], F32, tag="psS")
                nc.tensor.matmul(
                    wmm[:, :128], lhsT=zb[0:64, :], rhs=zb[0:64, :],
                    start=True, stop=True,
                )

            def qk_src(h):
                return qkT_d[h]

            def emit_body():
                # Per-head Q^T / K^T: [66, 2, s] (dim1: 0=Q^T, 1=K^T; rows
                # 64-65 are the Schraudolph bias constants).
                qks = []
                for h in range(HPC):
                    qk = wq.tile([HD + 2, 2, s], BF16, tag=f"qkT{h}")
                    qks.append(qk)
                v_sb = wq.tile([128, KS, HPC * (HD + 1)], BF16, tag="vsb")
                nm_sb = wq.tile([128, KS, s], BF16, tag="nm")
                KH = KS // 2
                # All input DMAs ride the SP HWDGE queue (SP has no compute,
                # so ring-full stalls never block a compute sequencer; gpsimd
                # dma_start is SWDGE and would burn Pool engine time). Pieces
                # are ordered by first use; phases run q-group-major, so mask
                # q-group 1 is not needed until ~halfway through the kernel.
                nm8p = {}
                for (g, ks) in i_pieces:
                    t = wq.tile([128, QG], FP8E5, tag=f"nm8_{g}_{ks}",
                                name=f"nm8_{g}_{ks}")
                    nm8p[(g, ks)] = t
                nm8mp = {}
                for (g, ks) in sorted(fp8m_pieces | fp8zx_pieces):
                    t = wq.tile([128, QG], FP8E4, tag=f"nm8m_{g}_{ks}",
                                name=f"nm8m_{g}_{ks}")
                    nm8mp[(g, ks)] = t

                def nm_piece(ks, g):
                    if (g, ks) in fp8m_pieces:
                        nc.sync.dma_start(
                            out=nm8mp[(g, ks)],
                            in_=nm8m_view[:, ks, g * QG:(g + 1) * QG],
                        )
                    elif (g, ks) in az_pieces:
                        nc.sync.dma_start(
                            out=nm_sb[:, ks, g * QG:(g + 1) * QG],
                            in_=nm_view[:, ks, g * QG:(g + 1) * QG],
                        )

                def nm8_piece(ks, g):
                    nc.sync.dma_start(
                        out=nm8p[(g, ks)],
                        in_=nm8_view[:, ks, g * QG:(g + 1) * QG],
                    )

                # Heads 0/1 split by channel so head 0's slices land first.
                # Head 1 is not consumed until phase 1 (~10us in), so its two
                # DMAs are deferred into the phase-0 mask stream (after piece
                # QK1_POS) - every earlier mask strip arrives ~1.6us sooner.
                def qk1_dmas():
                    nc.scalar.dma_start(
                        out=qks[1][:, 0, :], in_=qk_src(1)[:, 0, :]
                    )
                    nc.sync.dma_start(
                        out=qks[1][:, 1, :], in_=qk_src(1)[:, 1, :]
                    )

                nc.scalar.dma_start(out=qks[0][:, 0, :], in_=qk_src(0)[:, 0, :])
                nc.sync.dma_start(out=qks[0][:, 1, :], in_=qk_src(0)[:, 1, :])
                if QK1_POS == 0 or KS <= 8:
                    qk1_dmas()
                # Phase-0 mask pieces in consumption order: I strips need only
                # the 1-byte fp8 piece now (bf16 copy deferred until after v).
                ph0 = patterns[0]
                deferred = []
                for ks in range(KS):
                    if QK1_POS != 0 and KS > 8 and ks == QK1_POS:
                        qk1_dmas()
                    if (0, ks) in fp8zx_pieces:
                        # Early fp8 copy for the Z phases; bf16 deferred.
                        nc.sync.dma_start(
                            out=nm8mp[(0, ks)],
                            in_=nm8m_view[:, ks, 0:QG],
                        )
                        deferred.append(("nm", ks))
                    elif ph0[ks % len(ph0)] == "I":
                        if (0, ks) in i_pieces:
                            nm8_piece(ks, 0)
                        if (0, ks) in az_pieces or (0, ks) in fp8m_pieces:
                            deferred.append(("nm", ks))
                    else:
                        if (0, ks) in az_pieces or (0, ks) in fp8m_pieces:
                            nm_piece(ks, 0)
                        if (0, ks) in i_pieces:
                            deferred.append(("nm8", ks))
                # Deferred bf16 copies with early deadlines (ks < 8, needed by
                # phase 1 at ~11us) go before v; the rest after.
                for kind, ks in deferred:
                    if ks < 8:
                        (nm_piece if kind == "nm" else nm8_piece)(ks, 0)
                nc.sync.dma_start(out=v_sb[:, :KH], in_=v_view[:, :KH])
                nc.sync.dma_start(out=v_sb[:, KH:], in_=v_view[:, KH:])
                for kind, ks in deferred:
                    if ks >= 8:
                        (nm_piece if kind == "nm" else nm8_piece)(ks, 0)
                for h in range(2, HPC):
                    nc.sync.dma_start(out=qks[h], in_=qk_src(h))
                for g in range(1, NQG):
                    for ks in range(KS):
                        if (g, ks) in i_pieces:
                            nm8_piece(ks, g)
                        if (g, ks) in az_pieces or (g, ks) in fp8m_pieces:
                            nm_piece(ks, g)


                out_asm = opool.tile([128, KS, HPC * HD], BF16)

                # q-group-major phase order: the first HPC phases only touch
                # mask q-group 0, giving the mask DMA stream headroom.
                phases = [(h, g) for g in range(NQG) for h in range(HPC)]

                def emit_av_chunk(ph, c, av_state):
                    """AV matmuls for q-chunk c of phase ph, plus group
                    finalize (reciprocal + normalize) every CPG chunks.

                    Strips are read in eviction-completion order (Z first,
                    then A, then P): the last strips read are the ones whose
                    masks lag past the phase boundary, so the PE never waits
                    on a straggling Pool/DVE mask with work still in hand."""
                    h, g = ph
                    ats = av_state["ats"]
                    order = av_state["order"]
                    if c % CPG == 0:
                        av_state["psO"] = psO_pool.tile(
                            [128, CPG, 128], F32, tag="psO", name="psO"
                        )
                    psO = av_state["psO"]
                    for i, ks in enumerate(order):
                        _dbg(nc.tensor.matmul(
                            psO[:, c % CPG, 0:HD + 1],
                            lhsT=ats[ks][:, c * 128:(c + 1) * 128],
                            rhs=v_sb[:, ks, h * (HD + 1):(h + 1) * (HD + 1)],
                            start=(i == 0),
                            stop=(i == KS - 1),
                        ), f"AV h{h}g{g} c{c} ks{ks}")
                    fe = av_state.get("fin_every", CPG)
                    if c % fe == fe - 1:
                        fin = (h, g, c - (fe - 1), fe, psO)
                        if c == NCH - 1 and av_state.get("defer_last"):
                            av_state["deferred"] = fin
                        else:
                            emit_finalize(fin, av_state.get("norm_engine"))

                def emit_finalize(fin, norm_engine=None):
                    h, g, c0, fe, psO = fin
                    qc0 = g * NCH + c0
                    eng = norm_engine if norm_engine is not None else (
                        nc.gpsimd if NORM_POOL else nc.vector)
                    if USE_DIV:
                        # Single fused normalize: out = psO[:, :, 0:HD] / Z
                        # (column HD broadcast) - no separate reciprocal.
                        _dbg(eng.tensor_tensor(
                            out_asm[:, qc0:qc0 + fe, h * HD:(h + 1) * HD],
                            psO[:, c0 % CPG:c0 % CPG + fe, 0:HD],
                            psO[:, c0 % CPG:c0 % CPG + fe,
                                HD:HD + 1].to_broadcast([128, fe, HD]),
                            mybir.AluOpType.divide,
                        ), f"norm h{h}g{g} c{c0}")
                        return
                    rec = spool.tile([128, fe], F32, tag="rec", name="rec")
                    _dbg(nc.vector.reciprocal(
                        rec, psO[:, c0 % CPG:c0 % CPG + fe, HD]),
                         f"recip h{h}g{g} c{c0}")
                    _dbg(eng.tensor_mul(
                        out_asm[:, qc0:qc0 + fe, h * HD:(h + 1) * HD],
                        psO[:, c0 % CPG:c0 % CPG + fe, 0:HD],
                        rec.to_broadcast([128, fe, HD]),
                    ), f"norm h{h}g{g} c{c0}")
                    if h == HPC - 1:
                        nc.sync.dma_start(
                            out=out_view[:, qc0:qc0 + fe, :],
                            in_=out_asm[:, qc0:qc0 + fe, :],
                        )

                prev = None  # (phase, {"ats": [...]}) awaiting AV
                pending_fin = []
                unit = 0  # global eviction-unit counter (for path pattern)
                for ph in phases:
                    h, g = ph
                    q0 = g * QG
                    ats = []
                    paths = []
                    for ks in range(KS):
                        # QK for this unit
                        pat = patterns[(unit // KS) % len(patterns)]
                        path = pat[ks % len(pat)]
                        # Pure-Z strips contract over 66 rows: rows 64/65 add
                        # the Schraudolph bias B' to every score.
                        kb = (HD + 2 if path == "Z" and (g, ks) in fp8z_all
                              else HD)
                        psS = psS_pool.tile([128, QG], F32, tag="psS")
                        for qc in range(QG // 512):
                            _dbg(nc.tensor.matmul(
                                psS[:, qc * 512:(qc + 1) * 512],
                                lhsT=qks[h][0:kb, 1,
                                            ks * 128:(ks + 1) * 128],
                                rhs=qks[h][0:kb, 0,
                                           q0 + qc * 512:q0 + (qc + 1) * 512],
                                start=True,
                                stop=(path != "I"),
                            ), f"QK h{h}g{g} ks{ks}")
                        if path == "I":
                            # Mask-add on the PE: psS += I^T @ nm8 via a plain
                            # fp8 matmul (~213ns per 512 columns).
                            for qc in range(QG // 512):
                                _dbg(nc.tensor.matmul(
                                    psS[:, qc * 512:(qc + 1) * 512],
                                    lhsT=ident8[:, 0, :],
                                    rhs=nm8p[(g, ks)][:,
                                                      qc * 512:(qc + 1) * 512],
                                    start=False,
                                    stop=True,
                                ), f"maskI h{h}g{g} ks{ks}")
                        # Chunks ride units 4..~12: late enough that the
                        # previous phase's last evictions have drained, early
                        # enough that attn slots recycle before phase p+2.
                        if ks == 1 and pending_fin:
                            emit_finalize(pending_fin.pop())
                        if prev is not None:
                            start = AV_START if KS > 8 else 1
                            den = max(KS - start - 2, 1)
                            for c in range(NCH):
                                if min(start + c * den // NCH, KS - 1) == ks:
                                    emit_av_chunk(prev[0], c, prev[1])
                        # Eviction: psS -> masked bf16 attn tile
                        at = apool.tile([128, QG], BF16, tag="at")
                        nm_slice = nm_sb[:, ks, q0:q0 + QG]
                        unit += 1
                        if path == "I":
                            # Mask already added in PSUM; exp with bias ln(A')
                            # scales the weights to match the other paths.
                            _dbg(nc.scalar.activation(
                                at, psS, mybir.ActivationFunctionType.Exp,
                                scale=0.125, bias=lnap[:],
                            ), f"expI h{h}g{g} ks{ks}")
                        elif path == "Z" and (g, ks) in fp8z_all:
                            # Bias pre-added by the QK rows; fp8 {1,0} mask.
                            _dbg(nc.vector.scalar_tensor_tensor(
                                at[:].bitcast(I16),
                                psS[:],
                                A_PRIME,
                                nm8mp[(g, ks)],
                                mybir.AluOpType.mult,
                                mybir.AluOpType.mult,
                            ), f"STT8 h{h}g{g} ks{ks}")
                        elif path == "Z":
                            _dbg(nc.vector.scalar_tensor_tensor(
                                at[:].bitcast(I16),
                                psS[:],
                                B_PRIME,
                                nm_slice,
                                mybir.AluOpType.add,
                                mybir.AluOpType.mult,
                            ), f"STT h{h}g{g} ks{ks}")
                        elif path == "W":
                            _dbg(nc.gpsimd.scalar_tensor_tensor(
                                at[:].bitcast(I16),
                                psS[:],
                                B_PRIME,
                                nm_slice,
                                mybir.AluOpType.add,
                                mybir.AluOpType.mult,
                            ), f"STTW h{h}g{g} ks{ks}")
                        elif path in ("S", "T"):
                            # Split eviction: fast engines take cols [0:HF)
                            # (read by AV chunks 0..NCH/2-1, emitted first);
                            # Pool STT takes [HF:QG).
                            HF = QG // 2
                            if path == "S":
                                _dbg(nc.vector.scalar_tensor_tensor(
                                    at[:, 0:HF].bitcast(I16),
                                    psS[:, 0:HF],
                                    B_PRIME,
                                    nm_slice[:, 0:HF],
                                    mybir.AluOpType.add,
                                    mybir.AluOpType.mult,
                                ), f"STTh h{h}g{g} ks{ks}")
                            else:
                                _dbg(nc.scalar.activation(
                                    at[:, 0:HF], psS[:, 0:HF],
                                    mybir.ActivationFunctionType.Exp,
                                    scale=0.125,
                                ), f"expTh h{h}g{g} ks{ks}")
                                _dbg(nc.vector.tensor_mul(
                                    at[:, 0:HF], at[:, 0:HF],
                                    nm_slice[:, 0:HF]),
                                    f"maskTh h{h}g{g} ks{ks}")
                            _dbg(nc.gpsimd.scalar_tensor_tensor(
                                at[:, HF:QG].bitcast(I16),
                                psS[:, HF:QG],
                                B_PRIME,
                                nm_slice[:, HF:QG],
                                mybir.AluOpType.add,
                                mybir.AluOpType.mult,
                            ), f"STTWh h{h}g{g} ks{ks}")
                        elif path == "A":
                            _dbg(nc.scalar.activation(
                                at, psS, mybir.ActivationFunctionType.Exp,
                                scale=0.125,
                            ), f"expA h{h}g{g} ks{ks}")
                            _dbg(nc.vector.tensor_mul(at, at, nm_slice),
                                 f"maskA h{h}g{g} ks{ks}")
                        elif (g, ks) in fp8p_pieces:
                            # P-path with 1-byte mask: x A' folded into the
                            # exp bias, Pool multiplies by fp8 {1, 0}.
                            _dbg(nc.scalar.activation(
                                at, psS, mybir.ActivationFunctionType.Exp,
                                scale=0.125, bias=lnap[:],
                            ), f"expP8 h{h}g{g} ks{ks}")
                            _dbg(nc.gpsimd.tensor_mul(
                                at, at, nm8mp[(g, ks)]),
                                 f"maskP8 h{h}g{g} ks{ks}")
                        else:
                            _dbg(nc.scalar.activation(
                                at, psS, mybir.ActivationFunctionType.Exp,
                                scale=0.125,
                            ), f"expP h{h}g{g} ks{ks}")
                            _dbg(nc.gpsimd.tensor_mul(at, at, nm_slice),
                                 f"maskP h{h}g{g} ks{ks}")
                        ats.append(at)
                        paths.append(path)
                    rank = {"I": 0, "Z": 1, "S": 2, "T": 3, "A": 4, "W": 5,
                            "P": 6}
                    order = sorted(range(KS), key=lambda k: (rank[paths[k]], k))
                    if prev is not None and prev[1].get("deferred"):
                        pending_fin.append(prev[1]["deferred"])
                    prev = (ph, {"ats": ats, "order": order,
                                 "defer_last": KS > 8})
                # Tail: AV of the final phase. Finalize every 2 chunks (and
                # DMA out per finalize) so the drain chain after the last AV
                # matmul is short.
                prev[1]["defer_last"] = False
                prev[1]["fin_every"] = TAIL_FE
                for fin in pending_fin:
                    emit_finalize(fin)
                for c in range(NCH):
                    emit_av_chunk(prev[0], c, prev[1])

            for _ in range(reps):
                emit_body()
    nc.compile()
    return nc


_CACHE = {}


def _get_nc():
    if "nc" not in _CACHE:
        _CACHE["nc"] = build_program()
    return _CACHE["nc"]


def make_in_maps(q, k, v, mask, s=S):
    """Shard full inputs into 8 per-core input maps (host-side layout prep)."""
    q = np.asarray(q, dtype=np.float32)
    k = np.asarray(k, dtype=np.float32)
    v = np.asarray(v, dtype=np.float32)
    mask = np.asarray(mask)
    nh = q.shape[-1] // HD
    in_maps = []
    for c in range(NCORES):
        b, g = divmod(c, NCORES // B)
        h0 = HPC * g
        qs = q[b].reshape(s, nh, HD)[:, h0:h0 + HPC, :]      # [s, HPC, 64]
        ks_ = k[b].reshape(s, nh, HD)[:, h0:h0 + HPC, :]
        qkT = np.empty((HPC, HD + 2, 2, s), ml_dtypes.bfloat16)
        qkT[:, 0:HD, 0, :] = qs.transpose(1, 2, 0)
        qkT[:, 0:HD, 1, :] = ks_.transpose(1, 2, 0)
        # Schraudolph bias rows: Q-side 1.0; K-side 728.0 and -0.25292969
        # (bf16-exact; their sum ~= B_PRIME).
        qkT[:, HD:HD + 2, 0, :] = 1.0
        qkT[:, HD, 1, :] = 728.0
        qkT[:, HD + 1, 1, :] = -0.25292969
        vh = v[b, :, h0 * HD:(h0 + HPC) * HD].reshape(s, HPC, HD)
        vc = np.concatenate(
            [vh, np.ones((s, HPC, 1), np.float32)], axis=2
        ).reshape(s, HPC * (HD + 1)).astype(ml_dtypes.bfloat16)
        mT = mask[b].T
        nmT = (np.float32(A_PRIME) * (~mT).astype(np.float32)).astype(
            ml_dtypes.bfloat16
        )
        nm8 = (np.float32(MASK_BIAS) * mT.astype(np.float32)).astype(
            ml_dtypes.float8_e5m2
        )
        nm8m = (~mT).astype(np.float32).astype(ml_dtypes.float8_e4m3fn)
        in_maps.append(
            {"qkT": qkT, "v": vc, "nmT": nmT, "nm8": nm8, "nm8m": nm8m}
        )
    return in_maps


def assemble_out(results, s=S, d=D):
    out = np.empty((B, s, d), np.float32)
    for c in range(NCORES):
        b, g = divmod(c, NCORES // B)
        out[b, :, g * HPC * HD:(g + 1) * HPC * HD] = results[c]["out"]
    return out


def kernel(q, k, v, mask):
    from concourse.bass_utils import run_bass_kernel_spmd

    nc = _get_nc()
    in_maps = make_in_maps(q, k, v, mask)
    res = run_bass_kernel_spmd(nc, in_maps, list(range(NCORES))).results
    return assemble_out(res)

